# revision 1
# baseline (speedup 1.0000x reference)
"""Trainium2 Bass kernel for nn_GATv2Base (gnn_message_passing).

Contract: kernel(**inputs) takes FULL unsharded inputs (same keys as
reference.setup_inputs()) and returns the FULL [32, 64] float32 output.

Sharding: 32 graphs -> 8 cores (4 graphs each, contiguous node ranges since
`batch` is sorted).  Edges (plus self-loops) are routed to the core owning
their dst node, sorted by dst, and packed into "spans" (<=127-node dst window,
2048 edge slots = 16 subgroups of 128 edges).  Node features live in a
span-major padded global layout so every per-span device address is static.
Layer 1 runs fully local (xl1 table computed replicated from x); between
layers one fp16 AllGather shares the xl2 table; the pooled per-graph MLP is
computed on the owning core.  Only the [4, 64] per-core outputs return to the
host.
"""

import os
import sys

import numpy as np

for _p in ("/opt/trn_rl_repo", "/root/.axon_site/_ro/trn_rl_repo"):
    if os.path.isdir(_p) and _p not in sys.path:
        sys.path.insert(0, _p)

import concourse.bass as bass
import concourse.bacc as bacc
import concourse.mybir as mybir
import concourse.tile as tile
from concourse import library_config
from concourse.bass import IndirectOffsetOnAxis
from concourse.bass_utils import run_bass_kernel_spmd

F32 = mybir.dt.float32
F16 = mybir.dt.float16
I32 = mybir.dt.int32
AF = mybir.ActivationFunctionType
ALU = mybir.AluOpType
AXX = mybir.AxisListType.X

N, E, H, C, NG = 50000, 800000, 4, 64, 32
HC = H * C
NCORES = 8
SLOT_H = 1152            # edge slots per table-half region (9 subgroups)
SPAN_EDGES = 2 * SLOT_H  # 2304 edge slots per span (18 subgroups of 128)
NSG = SPAN_EDGES // 128  # 18
NSG_H = SLOT_H // 128    # 9
SPAN_DST = 127           # dst window per span; slot 127 = pad marker
DEBUG = False
PHASE_LIMIT = 99  # debug: truncate program after phase N
OP_LIMIT = 99     # debug: truncate gat span body


# ----------------------------------------------------------------------------
# Host-side sharding / packing
# ----------------------------------------------------------------------------

def _host_prep(inp):
    x = np.asarray(inp["x"], dtype=np.float32)
    ei = np.asarray(inp["edge_index"], dtype=np.int32)
    ea_full = np.asarray(inp["edge_attr"], dtype=np.float32)[:, 0]
    batch = np.asarray(inp["batch"], dtype=np.int32)

    src0, dst0 = ei[0], ei[1]
    deg = np.maximum(np.bincount(dst0, minlength=N).astype(np.float64), 1.0)
    loop_attr = (
        np.bincount(dst0, weights=ea_full.astype(np.float64), minlength=N) / deg
    ).astype(np.float32)
    src = np.concatenate([src0, np.arange(N, dtype=np.int32)])
    dst = np.concatenate([dst0, np.arange(N, dtype=np.int32)])
    eattr = np.concatenate([ea_full, loop_attr]).astype(np.float32)

    gcounts = np.bincount(batch, minlength=NG)
    gstart = np.concatenate([[0], np.cumsum(gcounts)])
    core_n0 = np.array([gstart[4 * k] for k in range(NCORES)] + [N], dtype=np.int64)

    order = np.argsort(dst, kind="stable")
    src, dst, eattr = src[order], dst[order], eattr[order]
    edge_lo = np.searchsorted(dst, core_n0[:-1], "left")
    edge_hi = np.searchsorted(dst, core_n0[1:], "left")

    # src owner core (cores 0-3 -> table half A, 4-7 -> half B); stable
    # under span-count changes so it can drive packing.
    src_owner = np.searchsorted(core_n0[1:], src, "right")
    src_in_a = src_owner < (NCORES // 2)

    cores = []
    for k in range(NCORES):
        n0, n1 = int(core_n0[k]), int(core_n0[k + 1])
        s, e = int(edge_lo[k]), int(edge_hi[k])
        cd = dst[s:e]
        ca = src_in_a[s:e]
        nlocal = n1 - n0
        node_edge_start = np.searchsorted(cd, n0 + np.arange(nlocal + 1))
        cumA = np.concatenate([[0], np.cumsum(ca)])  # over edges
        spans = []
        b = 0
        while b < nlocal:
            bend = b
            while bend < nlocal and (bend - b) < SPAN_DST:
                e0, e1 = node_edge_start[b], node_edge_start[bend + 1]
                nA = cumA[e1] - cumA[e0]
                nB = (e1 - e0) - nA
                if nA > SLOT_H or nB > SLOT_H:
                    break
                bend += 1
            assert bend > b, "single node exceeds span edge capacity"
            spans.append(
                (b, bend - b, int(node_edge_start[b]), int(node_edge_start[bend]))
            )
            b = bend
        cores.append(
            dict(n0=n0, n1=n1, spans=spans, src=src[s:e], dst=cd, ea=eattr[s:e],
                 in_a=ca)
        )

    nspans = max(len(c["spans"]) for c in cores)
    rows_per_core = nspans * 128
    rows_total = NCORES * rows_per_core

    # global padded row per node
    node_row = np.zeros(N, dtype=np.int64)
    for k, c in enumerate(cores):
        for si, (b, nb, _, _) in enumerate(c["spans"]):
            nodes = np.arange(c["n0"] + b, c["n0"] + b + nb)
            node_row[nodes] = k * rows_per_core + si * 128 + (nodes - c["n0"] - b)

    # x padded, transposed, with ones row (for encoder rhs)
    x_pad = np.zeros((rows_total, 4), dtype=np.float32)
    x_pad[node_row] = x
    x_aug_T = np.concatenate(
        [x_pad.T, np.ones((1, rows_total), dtype=np.float32)], axis=0
    )  # [5, R]

    packs = []
    half_rows = rows_total // 2
    assert half_rows <= 32767, f"table half {half_rows} exceeds int16 index range"

    def wrap_idx16(vals):
        # vals: [SLOT] int -> wrapped [128, SLOT//16] int16 (16-part wrap,
        # replicated over the 8 q7 core groups)
        slot = len(vals)
        base = np.zeros((16, slot // 16), dtype=np.int16)
        i = np.arange(slot)
        base[i % 16, i // 16] = vals.astype(np.int16)
        return np.tile(base, (8, 1))

    for k, c in enumerate(cores):
        src_idx_a = np.zeros((nspans, 128, SLOT_H // 16), dtype=np.int16)
        src_idx_b = np.zeros((nspans, 128, SLOT_H // 16), dtype=np.int16)
        xr_idx = np.zeros((nspans, 128, SPAN_EDGES // 16), dtype=np.int16)
        dcol = np.full((nspans, 128, NSG), 127.0, dtype=np.float32)
        ea4 = np.zeros((nspans, 128, 4 * NSG), dtype=np.float16)
        ea_col = np.zeros((nspans, 128, NSG), dtype=np.float32)
        gmask = np.zeros((nspans, 128, 4), dtype=np.float16)
        for si, (b, nb, e0, e1) in enumerate(c["spans"]):
            ina = c["in_a"][e0:e1]
            esrc = node_row[c["src"][e0:e1]]
            edrel = (c["dst"][e0:e1] - c["n0"] - b).astype(np.int64)
            eea = c["ea"][e0:e1]
            # slots: A edges first (in region [0, SLOT_H)), then B edges at
            # [SLOT_H, 2*SLOT_H); pads keep idx 0 / drel 127 / ea 0
            ia = np.where(ina)[0]
            ib = np.where(~ina)[0]
            slots = np.empty(len(ina), dtype=np.int64)
            slots[ia] = np.arange(len(ia))
            slots[ib] = SLOT_H + np.arange(len(ib))
            av = np.zeros(SLOT_H, dtype=np.int64)
            av[:len(ia)] = esrc[ia]
            bv = np.zeros(SLOT_H, dtype=np.int64)
            bv[:len(ib)] = esrc[ib] - half_rows
            src_idx_a[si] = wrap_idx16(av)
            src_idx_b[si] = wrap_idx16(bv)
            xv = np.full(SPAN_EDGES, si * 128 + 127, dtype=np.int64)
            xv[slots] = si * 128 + edrel
            xr_idx[si] = wrap_idx16(xv)
            p, sg = slots % 128, slots // 128
            dcol[si, p, sg] = edrel.astype(np.float32)
            ea_col[si, p, sg] = eea
            for hh in range(4):
                ea4[si, p, sg * 4 + hh] = eea.astype(np.float16)
            nodes = np.arange(c["n0"] + b, c["n0"] + b + nb)
            gl = batch[nodes] - 4 * k
            gmask[si, np.arange(nb), gl] = np.float16(1.0)
        inv_cnt = np.zeros((4, 1), dtype=np.float32)
        for gg in range(4):
            cnt = max(int(gcounts[4 * k + gg]), 1)
            inv_cnt[gg, 0] = 1.0 / cnt
        packs.append(
            dict(
                src_idx_a=src_idx_a,
                src_idx_b=src_idx_b,
                xr_idx=xr_idx,
                dcol=dcol,
                ea4=ea4,
                ea_col=ea_col,
                gmask=gmask,
                inv_cnt=inv_cnt,
                own_cols=np.arange(
                    k * rows_per_core, (k + 1) * rows_per_core, dtype=np.int64
                ),
            )
        )
    return cores, packs, nspans, rows_per_core, rows_total, x_aug_T, node_row


# ----------------------------------------------------------------------------
# Device program
# ----------------------------------------------------------------------------

_PROGRAM_CACHE = {}


def _build_program(nspans, rows_total):
    rows_per_core = nspans * 128
    nblocks = rows_total // 128

    nc = bacc.Bacc()
    tcx = tile.TileContext(nc)

    # ---- external inputs (per core) ----
    def din(name, shape, dt):
        return nc.dram_tensor(name, shape, dt, kind="ExternalInput")

    t_xaugT = din("xaugT", [5, rows_total], F32)
    t_own_xaugT = din("own_xaugT", [5, rows_per_core], F32)
    t_enc_aug = din("enc_aug", [5, 64], F32)
    t_w1 = {}
    t_w2 = {}
    for L, tw in ((1, t_w1), (2, t_w2)):
        kdim = 65 if L == 1 else 257
        tw["wl_aug"] = din(f"wl{L}_aug", [kdim, HC], F16)
        tw["wr_aug"] = din(f"wr{L}_aug", [kdim, HC], F16)
        tw["att_row"] = din(f"att{L}_row", [128, HC], F16)
        tw["we_row"] = din(f"we{L}_row", [128, HC], F16)
        tw["bias_row"] = din(f"bias{L}_row", [128, HC], F16)
    I16 = mybir.dt.int16
    t_src_idx_a = din("src_idx_a", [nspans, 128, SLOT_H // 16], I16)
    t_src_idx_b = din("src_idx_b", [nspans, 128, SLOT_H // 16], I16)
    t_xr_idx = din("xr_idx", [nspans, 128, SPAN_EDGES // 16], I16)
    t_dcol = din("dcol", [nspans, 128, NSG], F32)
    t_ea4 = din("ea4", [nspans, 128, 4 * NSG], F16)
    t_ea_col = din("ea_col", [nspans, 128, NSG], F32)
    t_gmask = din("gmask", [nspans, 128, 4], F16)
    t_iota_row = din("iota_row", [128, 128], F16)
    t_inv_cnt = din("inv_cnt", [4, 1], F32)
    t_p1_aug = din("p1_aug", [257, 128], F32)
    t_ln_g = din("ln_g4", [4, 128], F32)
    t_ln_b = din("ln_b4", [4, 128], F32)
    t_p2_aug = din("p2_aug", [129, 64], F32)
    t_out = nc.dram_tensor("out", [4, 64], F32, kind="ExternalOutput")
    t_dbg = {}
    if DEBUG:
        for nm, shp, dt in (("v", [128, NSG * HC], F16), ("u", [128, NSG * HC], F16),
                            ("alpha", [128, 4 * NSG], F32), ("ex", [128, 4 * NSG], F32),
                            ("A", [128, HC], F32), ("den", [128, 4], F32),
                            ("m2", [128, NSG * HC], F16), ("S", [128, NSG * 128], F16),
                            ("hT", [128, HC], F16)):
            t_dbg[nm] = nc.dram_tensor("dbg_" + nm, shp, dt, kind="ExternalOutput")

    # ---- internal DRAM ----
    t_xl1 = nc.dram_tensor("xl1_tbl", [rows_total, HC], F16)
    t_xr1 = nc.dram_tensor("xr1_own", [rows_per_core, HC], F16)
    t_h1 = nc.dram_tensor("h1_own", [rows_per_core, HC], F16)
    t_xr2 = nc.dram_tensor("xr2_own", [rows_per_core, HC], F16)
    t_xl2_in = nc.dram_tensor("xl2_own_cc", [rows_per_core, HC], F16)
    t_xl2 = nc.dram_tensor("xl2_tbl", [rows_total, HC], F16, addr_space="Shared")

    from contextlib import ExitStack
    with tcx as tc, ExitStack() as es:
        # ------------------------------------------------------------------
        # constants in SBUF
        # ------------------------------------------------------------------
        cpool = es.enter_context(tc.tile_pool(name="consts", bufs=1))
        enc_aug = cpool.tile([5, 64], F32)
        nc.sync.dma_start(out=enc_aug[:], in_=t_enc_aug[:])
        iota_rep = cpool.tile([128, 128], F16)
        nc.sync.dma_start(out=iota_rep[:], in_=t_iota_row[:])
        reps = {}
        for L, tw in ((1, t_w1), (2, t_w2)):
            for nm in ("att_row", "we_row", "bias_row"):
                rep = cpool.tile([128, HC], F16, tag=f"rep{L}{nm}")
                nc.sync.dma_start(out=rep[:], in_=tw[nm][:])
                reps[(L, nm)] = rep
        ones_col = cpool.tile([1, 128], F16)
        nc.vector.memset(ones_col[:], 1.0)

        # ------------------------------------------------------------------
        # Phase 1: encoder + xl1 for ALL rows ; wl/wr weights resident
        # ------------------------------------------------------------------
        wpool = es.enter_context(tc.tile_pool(name="weights", bufs=1))
        wl1 = wpool.tile([65, HC], F16)
        wr1 = wpool.tile([65, HC], F16)
        nc.sync.dma_start(out=wl1[:], in_=t_w1["wl_aug"][:])
        nc.sync.dma_start(out=wr1[:], in_=t_w1["wr_aug"][:])

        def encode_block(pool, ppool, xaugT_ap):
            """xaugT_ap: [5, 128] f32 dram slice -> h0T_aug [65, 128] f16 tile"""
            xT = pool.tile([5, 128], F32, tag="xT")
            nc.sync.dma_start(out=xT[:], in_=xaugT_ap)
            h0psum = ppool.tile([64, 128], F32, tag="h0ps")
            nc.tensor.matmul(out=h0psum[:], lhsT=enc_aug[:], rhs=xT[:],
                             start=True, stop=True)
            h0T = pool.tile([65, 128], F16, tag="h0T")
            nc.scalar.activation(out=h0T[0:64, :], in_=h0psum[:], func=AF.Relu)
            nc.vector.tensor_copy(out=h0T[64:65, :], in_=ones_col[:])
            return h0T

        with tc.tile_pool(name="p1", bufs=3) as pool, \
             tc.tile_pool(name="p1ps", bufs=2, space="PSUM") as ppool:
            for blk in range(nblocks if PHASE_LIMIT >= 1 else 0):
                h0T = encode_block(pool, ppool, t_xaugT[:, blk * 128:(blk + 1) * 128])
                xlp = ppool.tile([128, HC], F32, tag="xlps")
                nc.tensor.matmul(out=xlp[:], lhsT=h0T[:], rhs=wl1[:],
                                 start=True, stop=True)
                xls = pool.tile([128, HC], F16, tag="xls")
                if blk % 2 == 0:
                    nc.vector.tensor_copy(out=xls[:], in_=xlp[:])
                else:
                    nc.scalar.copy(out=xls[:], in_=xlp[:])
                nc.sync.dma_start(
                    out=t_xl1[blk * 128:(blk + 1) * 128, :], in_=xls[:]
                )

            # own xr1
            for s in range(nspans if PHASE_LIMIT >= 1 else 0):
                h0T = encode_block(pool, ppool, t_own_xaugT[:, s * 128:(s + 1) * 128])
                xrp = ppool.tile([128, HC], F32, tag="xlps")
                nc.tensor.matmul(out=xrp[:], lhsT=h0T[:], rhs=wr1[:],
                                 start=True, stop=True)
                xrs = pool.tile([128, HC], F16, tag="xls")
                nc.vector.tensor_copy(out=xrs[:], in_=xrp[:])
                nc.sync.dma_start(
                    out=t_xr1[s * 128:(s + 1) * 128, :], in_=xrs[:]
                )

        # ------------------------------------------------------------------
        # GAT span loop (shared for both layers)
        # ------------------------------------------------------------------
        def gat_layer(L, xl_tbl, xr_tbl, h_sink):
            """h_sink(s, htile): consume flush output [128, HC] f16."""
            att_rep = reps[(L, "att_row")]
            we_rep = reps[(L, "we_row")]
            bias_rep = reps[(L, "bias_row")]
            with tc.tile_pool(name=f"g{L}", bufs=2) as pool, \
                 tc.tile_pool(name=f"g{L}b", bufs=3) as spool, \
                 tc.tile_pool(name=f"g{L}ps", bufs=2, space="PSUM") as ppool:
                half_rows = rows_total // 2
                for s in range(nspans):
                    sidxa = spool.tile([128, SLOT_H // 16], I16, tag="sidxa")
                    nc.sync.dma_start(out=sidxa[:], in_=t_src_idx_a[s, :, :])
                    sidxb = spool.tile([128, SLOT_H // 16], I16, tag="sidxb")
                    nc.sync.dma_start(out=sidxb[:], in_=t_src_idx_b[s, :, :])
                    xidx = spool.tile([128, SPAN_EDGES // 16], I16, tag="xidx")
                    nc.sync.dma_start(out=xidx[:], in_=t_xr_idx[s, :, :])
                    dcol = spool.tile([128, NSG], F32, tag="dcol")
                    nc.sync.dma_start(out=dcol[:], in_=t_dcol[s, :, :])
                    eac = spool.tile([128, NSG], F32, tag="eac")
                    nc.sync.dma_start(out=eac[:], in_=t_ea_col[s, :, :])
                    ea4 = spool.tile([128, 4 * NSG], F16, tag="ea4")
                    nc.sync.dma_start(out=ea4[:], in_=t_ea4[s, :, :])
                    xr_fl = spool.tile([128, HC], F16, tag="xrfl")
                    nc.sync.dma_start(
                        out=xr_fl[:], in_=xr_tbl[s * 128:(s + 1) * 128, :]
                    )
                    xrb = spool.tile([128, HC], F16, tag="xrb")
                    nc.vector.tensor_tensor(out=xrb[:], in0=xr_fl[:],
                                            in1=bias_rep[:], op=ALU.subtract)

                    # G = xl[src] (two half-table gathers), R = xr[dst],
                    # v = (we*ea + R) + G
                    G = pool.tile([128, NSG, HC], F16, tag="G")
                    nc.gpsimd.dma_gather(
                        G[:, 0:NSG_H, :], xl_tbl[0:half_rows, :], sidxa[:, :],
                        SLOT_H, SLOT_H, HC, single_packet=False,
                    )
                    nc.gpsimd.dma_gather(
                        G[:, NSG_H:NSG, :], xl_tbl[half_rows:, :], sidxb[:, :],
                        SLOT_H, SLOT_H, HC, single_packet=False,
                    )
                    R = pool.tile([128, NSG, HC], F16, tag="R")
                    nc.gpsimd.dma_gather(
                        R[:, :, :], xr_tbl[:, :], xidx[:, :],
                        SPAN_EDGES, SPAN_EDGES, HC, single_packet=False,
                    )
                    if OP_LIMIT < 2:
                        hOut = spool.tile([128, HC], F16, tag="hOut")
                        nc.vector.tensor_copy(out=hOut[:], in_=G[:, 0, :])
                        h_sink(s, hOut, pool, ppool)
                        continue
                    v = pool.tile([128, NSG, HC], F16, tag="v")
                    for sg in range(NSG):
                        nc.vector.scalar_tensor_tensor(
                            out=v[:, sg, :], in0=we_rep[:],
                            scalar=eac[:, sg:sg + 1], in1=R[:, sg, :],
                            op0=ALU.mult, op1=ALU.add,
                        )
                    nc.vector.tensor_tensor(
                        out=v[:, :, :], in0=v[:, :, :], in1=G[:, :, :], op=ALU.add
                    )

                    if OP_LIMIT < 3:
                        hOut = spool.tile([128, HC], F16, tag="hOut")
                        nc.vector.tensor_copy(out=hOut[:], in_=v[:, 0, :])
                        h_sink(s, hOut, pool, ppool)
                        continue
                    # u = lrelu(v), z = u*att, alpha = per-head sum
                    u = pool.tile([128, NSG, HC], F16, tag="u")
                    nc.scalar.activation(out=u[:, :, :], in_=v[:, :, :],
                                         func=AF.Lrelu, alpha=0.2)
                    z = pool.tile([128, NSG, HC], F16, tag="z")
                    nc.vector.tensor_tensor(
                        out=z[:, :, :], in0=u[:, :, :],
                        in1=att_rep[:].rearrange("p (o c) -> p o c", o=1).broadcast_to((128, NSG, HC)), op=ALU.mult
                    )
                    if OP_LIMIT < 4:
                        hOut = spool.tile([128, HC], F16, tag="hOut")
                        nc.vector.tensor_copy(out=hOut[:], in_=z[:, 0, :])
                        h_sink(s, hOut, pool, ppool)
                        continue
                    # per-head sums via binary fold tree (TT 2x beats reduce 1x)
                    zf = pool.tile([128, NSG, 4, 32], F16, tag="zf")
                    z4 = z[:].rearrange("p s (h c) -> p s h c", h=4)
                    nc.vector.tensor_tensor(
                        out=zf[:, :, :, :], in0=z4[:, :, :, 0:32],
                        in1=z4[:, :, :, 32:64], op=ALU.add,
                    )
                    w = 16
                    while w >= 2:
                        nc.vector.tensor_tensor(
                            out=zf[:, :, :, 0:w], in0=zf[:, :, :, 0:w],
                            in1=zf[:, :, :, w:2 * w], op=ALU.add,
                        )
                        w //= 2
                    alpha = spool.tile([128, 4 * NSG], F32, tag="alpha")
                    nc.vector.tensor_tensor(
                        out=alpha[:].rearrange("p (s h o) -> p s h o", h=4, o=1),
                        in0=zf[:, :, :, 0:1], in1=zf[:, :, :, 1:2], op=ALU.add,
                    )
                    exF = spool.tile([128, 4 * NSG], F32, tag="exF")
                    nc.scalar.activation(out=exF[:], in_=alpha[:], func=AF.Exp)
                    ex = spool.tile([128, 4 * NSG], F16, tag="ex")
                    nc.vector.tensor_copy(out=ex[:], in_=exF[:])
                    exea = spool.tile([128, 4 * NSG], F16, tag="exea")
                    nc.vector.tensor_tensor(out=exea[:], in0=ex[:], in1=ea4[:],
                                            op=ALU.mult)

                    if OP_LIMIT < 5:
                        hOut = spool.tile([128, HC], F16, tag="hOut")
                        nc.vector.tensor_copy(out=hOut[:], in_=exea[:, 0:HC//4].rearrange("p a -> p a").broadcast_to((128, 4*NSG)) if False else v[:, 0, :])
                        h_sink(s, hOut, pool, ppool)
                        continue
                    # m2 = ex * v (per head), S one-hot, agg matmuls
                    m2 = pool.tile([128, NSG, HC], F16, tag="m2")
                    S = pool.tile([128, NSG, 128], F16, tag="S")
                    accM = ppool.tile([128, HC], F32, tag="accM")
                    accE = ppool.tile([128, 4], F32, tag="accE")
                    accX = ppool.tile([128, 4], F32, tag="accX")
                    for sg in range(NSG):
                        for hh in range(4):
                            nc.vector.tensor_scalar(
                                out=m2[:, sg, hh * C:(hh + 1) * C],
                                in0=v[:, sg, hh * C:(hh + 1) * C],
                                scalar1=exF[:, sg * 4 + hh:sg * 4 + hh + 1],
                                scalar2=None, op0=ALU.mult,
                            )
                        nc.vector.tensor_scalar(
                            out=S[:, sg, :], in0=iota_rep[:],
                            scalar1=dcol[:, sg:sg + 1], scalar2=None,
                            op0=ALU.is_equal,
                        )
                        nc.tensor.matmul(out=accM[:], lhsT=S[:, sg, :],
                                         rhs=m2[:, sg, :], start=(sg == 0),
                                         stop=(sg == NSG - 1))
                        nc.tensor.matmul(out=accE[:], lhsT=S[:, sg, :],
                                         rhs=ex[:, sg * 4:sg * 4 + 4],
                                         start=(sg == 0), stop=(sg == NSG - 1))
                        nc.tensor.matmul(out=accX[:], lhsT=S[:, sg, :],
                                         rhs=exea[:, sg * 4:sg * 4 + 4],
                                         start=(sg == 0), stop=(sg == NSG - 1))

                    if OP_LIMIT < 6:
                        hOut = spool.tile([128, HC], F16, tag="hOut")
                        nc.vector.tensor_copy(out=hOut[:], in_=accM[:])
                        h_sink(s, hOut, pool, ppool)
                        continue
                    # flush: h = relu(acc/den - xr' - we*(eaden/den))
                    A = spool.tile([128, HC], F32, tag="A")
                    nc.scalar.copy(out=A[:], in_=accM[:])
                    den = spool.tile([128, 4], F32, tag="den")
                    nc.vector.tensor_scalar(
                        out=den[:], in0=accE[:], scalar1=1e-30,
                        scalar2=None, op0=ALU.add,
                    )
                    rden = spool.tile([128, 4], F32, tag="rden")
                    nc.vector.reciprocal(out=rden[:], in_=den[:])
                    eaden = spool.tile([128, 4], F32, tag="eaden")
                    nc.vector.tensor_copy(out=eaden[:], in_=accX[:])
                    eard_n = spool.tile([128, 4], F32, tag="eardn")
                    for hh in range(4):
                        nc.vector.tensor_scalar(
                            out=eard_n[:, hh:hh + 1],
                            in0=eaden[:, hh:hh + 1],
                            scalar1=rden[:, hh:hh + 1], scalar2=-1.0,
                            op0=ALU.mult, op1=ALU.mult,
                        )
                    hT = spool.tile([128, HC], F16, tag="hT")
                    for hh in range(4):
                        blks = slice(hh * C, (hh + 1) * C)
                        nc.vector.scalar_tensor_tensor(
                            out=hT[:, blks], in0=A[:, blks],
                            scalar=rden[:, hh:hh + 1], in1=xrb[:, blks],
                            op0=ALU.mult, op1=ALU.subtract,
                        )
                        nc.vector.scalar_tensor_tensor(
                            out=hT[:, blks], in0=we_rep[:, blks],
                            scalar=eard_n[:, hh:hh + 1], in1=hT[:, blks],
                            op0=ALU.mult, op1=ALU.add,
                        )
                    hOut = spool.tile([128, HC], F16, tag="hOut")
                    nc.scalar.activation(out=hOut[:], in_=hT[:], func=AF.Relu)
                    if DEBUG and L == 1 and s == 0:
                        nc.sync.dma_start(out=t_dbg["v"][:], in_=v[:].rearrange("p a b -> p (a b)"))
                        nc.sync.dma_start(out=t_dbg["u"][:], in_=u[:].rearrange("p a b -> p (a b)"))
                        nc.sync.dma_start(out=t_dbg["alpha"][:], in_=alpha[:])
                        nc.sync.dma_start(out=t_dbg["ex"][:], in_=exF[:])
                        nc.sync.dma_start(out=t_dbg["A"][:], in_=A[:])
                        nc.sync.dma_start(out=t_dbg["den"][:], in_=den[:])
                        nc.sync.dma_start(out=t_dbg["m2"][:], in_=m2[:].rearrange("p a b -> p (a b)"))
                        nc.sync.dma_start(out=t_dbg["S"][:], in_=S[:].rearrange("p a b -> p (a b)"))
                        nc.sync.dma_start(out=t_dbg["hT"][:], in_=hT[:])
                    h_sink(s, hOut, pool, ppool)

        # layer 1: sink writes h1 to DRAM
        def h1_sink(s, hOut, pool, ppool):
            nc.sync.dma_start(out=t_h1[s * 128:(s + 1) * 128, :], in_=hOut[:])

        if PHASE_LIMIT >= 2:
            gat_layer(1, t_xl1, t_xr1, h1_sink)
        if PHASE_LIMIT < 5:
            with tc.tile_pool(name="dummyout", bufs=1) as dpool:
                dz = dpool.tile([4, 64], F32)
                nc.vector.memset(dz[:], 0.0)
                nc.sync.dma_start(out=t_out[:], in_=dz[:])

        # ------------------------------------------------------------------
        # Phase 4: xl2/xr2 from h1 (own spans)
        # ------------------------------------------------------------------
        w2_tiles = {}
        for nm in ("wl_aug", "wr_aug") if PHASE_LIMIT >= 3 else ():
            a = wpool.tile([128, HC], F16, tag=f"{nm}a")
            b = wpool.tile([128, HC], F16, tag=f"{nm}b")
            cbias = wpool.tile([1, HC], F16, tag=f"{nm}c")
            nc.sync.dma_start(out=a[:], in_=t_w2[nm][0:128, :])
            nc.sync.dma_start(out=b[:], in_=t_w2[nm][128:256, :])
            nc.sync.dma_start(out=cbias[:], in_=t_w2[nm][256:257, :])
            w2_tiles[nm] = (a, b, cbias)
        with tc.tile_pool(name="p4", bufs=3) as pool, \
             tc.tile_pool(name="p4ps", bufs=2, space="PSUM") as ppool:
            for s in range(nspans if PHASE_LIMIT >= 3 else 0):
                h1T0 = pool.tile([128, 128], F16, tag="h1T0")
                h1T1 = pool.tile([128, 128], F16, tag="h1T1")
                nc.sync.dma_start(
                    out=h1T0[:], in_=t_h1[s * 128:(s + 1) * 128, 0:128],
                    transpose=True,
                )
                nc.sync.dma_start(
                    out=h1T1[:], in_=t_h1[s * 128:(s + 1) * 128, 128:256],
                    transpose=True,
                )
                for nm, sink in (("wl_aug", t_xl2_in), ("wr_aug", t_xr2)):
                    wa, wb, wc = w2_tiles[nm]
                    ps = ppool.tile([128, HC], F32, tag="ps")
                    nc.tensor.matmul(out=ps[:], lhsT=h1T0[:], rhs=wa[:],
                                     start=True, stop=False)
                    nc.tensor.matmul(out=ps[:], lhsT=h1T1[:], rhs=wb[:],
                                     start=False, stop=False)
                    nc.tensor.matmul(out=ps[:], lhsT=ones_col[:],
                                     rhs=wc[:], start=False, stop=True)
                    xs = pool.tile([128, HC], F16, tag="xs")
                    nc.vector.tensor_copy(out=xs[:], in_=ps[:])
                    nc.sync.dma_start(out=sink[s * 128:(s + 1) * 128, :], in_=xs[:])

        # ------------------------------------------------------------------
        # Phase 5: AllGather xl2
        # ------------------------------------------------------------------
        if PHASE_LIMIT >= 4:
            nc.gpsimd.collective_compute(
                "AllGather",
            ALU.bypass,
                replica_groups=[list(range(NCORES))],
                ins=[t_xl2_in.ap().opt()],
                outs=[t_xl2.ap().opt()],
            )

        # ------------------------------------------------------------------
        # Phase 6: GAT layer 2 with fused pooling
        # ------------------------------------------------------------------
        gpool_ps = es.enter_context(tc.tile_pool(name="gpool_ps", bufs=1, space="PSUM"))
        gpsum = gpool_ps.tile([4, HC], F32)

        def h2_sink(s, hOut, pool, ppool):
            gm = pool.tile([128, 4], F16, tag="gm")
            nc.sync.dma_start(out=gm[:], in_=t_gmask[s, :, :])
            nc.tensor.matmul(out=gpsum[:], lhsT=gm[:], rhs=hOut[:],
                             start=(s == 0), stop=(s == nspans - 1))

        if PHASE_LIMIT >= 5:
            gat_layer(2, t_xl2, t_xr2, h2_sink)
        else:
            nc.vector.memset(gpsum[:], 0.0)

        # ------------------------------------------------------------------
        # Phase 7: pooling -> MLP -> out
        # ------------------------------------------------------------------
        with tc.tile_pool(name="mlp", bufs=1) as pool, \
             tc.tile_pool(name="mlp_ps", bufs=2, space="PSUM") as ppool:
          if PHASE_LIMIT >= 5:
            icnt = pool.tile([4, 1], F32)
            nc.sync.dma_start(out=icnt[:], in_=t_inv_cnt[:])
            g = pool.tile([4, HC], F32)
            nc.vector.tensor_scalar(out=g[:], in0=gpsum[:], scalar1=icnt[:, 0:1],
                                    scalar2=None, op0=ALU.mult)
            p1a = pool.tile([128, 128], F32)
            p1b = pool.tile([128, 128], F32)
            p1c = pool.tile([1, 128], F32)
            nc.sync.dma_start(out=p1a[:], in_=t_p1_aug[0:128, :])
            nc.sync.dma_start(out=p1b[:], in_=t_p1_aug[128:256, :])
            nc.sync.dma_start(out=p1c[:], in_=t_p1_aug[256:257, :])
            p2a = pool.tile([128, 64], F32)
            p2c = pool.tile([1, 64], F32)
            nc.sync.dma_start(out=p2a[:], in_=t_p2_aug[0:128, :])
            nc.sync.dma_start(out=p2c[:], in_=t_p2_aug[128:129, :])
            lng = pool.tile([4, 128], F32)
            nc.sync.dma_start(out=lng[:], in_=t_ln_g[:])
            lnb = pool.tile([4, 128], F32)
            nc.sync.dma_start(out=lnb[:], in_=t_ln_b[:])
            ident = pool.tile([128, 128], F32)
            from concourse.masks import make_identity
            make_identity(nc, ident[:])

            # gT via PE transpose (two halves)
            gT = pool.tile([128, 8], F32)  # [:, 0:4] = cols 0:128, [:, 4:8] = 128:256
            for half in range(2):
                tp = ppool.tile([128, 128], F32, tag="tp")
                nc.tensor.transpose(
                    out=tp[:, 0:4], in_=g[:, half * 128:(half + 1) * 128],
                    identity=ident[0:4, 0:4],
                )
                nc.vector.tensor_copy(out=gT[:, half * 4:half * 4 + 4],
                                      in_=tp[:, 0:4])
            onesg = pool.tile([1, 4], F32)
            nc.vector.memset(onesg[:], 1.0)
            z1p = ppool.tile([4, 128], F32, tag="z1p")
            nc.tensor.matmul(out=z1p[:], lhsT=gT[:, 0:4], rhs=p1a[:],
                             start=True, stop=False)
            nc.tensor.matmul(out=z1p[:], lhsT=gT[:, 4:8], rhs=p1b[:],
                             start=False, stop=False)
            nc.tensor.matmul(out=z1p[:], lhsT=onesg[:], rhs=p1c[:],
                             start=False, stop=True)
            z1 = pool.tile([4, 128], F32)
            nc.vector.tensor_copy(out=z1[:], in_=z1p[:])
            # layernorm over free dim (128)
            mu = pool.tile([4, 1], F32)
            nc.vector.reduce_sum(out=mu[:], in_=z1[:], axis=AXX)
            nc.vector.tensor_scalar(out=mu[:], in0=mu[:], scalar1=1.0 / 128,
                                    scalar2=None, op0=ALU.mult)
            zc = pool.tile([4, 128], F32)
            nc.vector.tensor_scalar(out=zc[:], in0=z1[:], scalar1=mu[:, 0:1],
                                    scalar2=None, op0=ALU.subtract)
            sq = pool.tile([4, 128], F32)
            nc.vector.tensor_tensor(out=sq[:], in0=zc[:], in1=zc[:], op=ALU.mult)
            var = pool.tile([4, 1], F32)
            nc.vector.reduce_sum(out=var[:], in_=sq[:], axis=AXX)
            nc.vector.tensor_scalar(out=var[:], in0=var[:], scalar1=1.0 / 128,
                                    scalar2=1e-5, op0=ALU.mult, op1=ALU.add)
            std = pool.tile([4, 1], F32)
            nc.scalar.activation(out=std[:], in_=var[:], func=AF.Sqrt)
            rstd = pool.tile([4, 1], F32)
            nc.vector.reciprocal(out=rstd[:], in_=std[:])
            zn = pool.tile([4, 128], F32)
            nc.vector.tensor_scalar(out=zn[:], in0=zc[:], scalar1=rstd[:, 0:1],
                                    scalar2=None, op0=ALU.mult)
            nc.vector.tensor_tensor(out=zn[:], in0=zn[:], in1=lng[:], op=ALU.mult)
            nc.vector.tensor_tensor(out=zn[:], in0=zn[:], in1=lnb[:], op=ALU.add)
            nc.scalar.activation(out=zn[:], in_=zn[:], func=AF.Relu)
            # z2 = relu(zn @ p2 + b2)
            znT = pool.tile([128, 4], F32)
            tp2 = ppool.tile([128, 128], F32, tag="tp")
            nc.tensor.transpose(out=tp2[:, 0:4], in_=zn[:], identity=ident[0:4, 0:4])
            nc.vector.tensor_copy(out=znT[:], in_=tp2[:, 0:4])
            z2p = ppool.tile([4, 64], F32, tag="z2p")
            nc.tensor.matmul(out=z2p[:], lhsT=znT[:], rhs=p2a[:],
                             start=True, stop=False)
            nc.tensor.matmul(out=z2p[:], lhsT=onesg[:], rhs=p2c[:],
                             start=False, stop=True)
            zout = pool.tile([4, 64], F32)
            nc.scalar.activation(out=zout[:], in_=z2p[:], func=AF.Relu)
            nc.sync.dma_start(out=t_out[:], in_=zout[:])

    nc.finalize()
    return nc


# ----------------------------------------------------------------------------
# Entry point
# ----------------------------------------------------------------------------

def _pack_inputs(inp, cores, packs, nspans, rows_per_core, rows_total, x_aug_T):
    f16 = np.float16
    iota_row = np.broadcast_to(
        np.arange(128, dtype=f16)[None, :], (128, 128)
    ).copy()
    in_maps = []
    for k in range(NCORES):
        p = packs[k]
        m = {
            "xaugT": x_aug_T.astype(np.float32),
            "own_xaugT": np.ascontiguousarray(
                x_aug_T[:, p["own_cols"]]
            ).astype(np.float32),
            "enc_aug": np.concatenate(
                [np.asarray(inp["enc_w"], np.float32),
                 np.asarray(inp["enc_b"], np.float32)[None, :]], 0
            ),
            "src_idx_a": p["src_idx_a"],
            "src_idx_b": p["src_idx_b"],
            "xr_idx": p["xr_idx"],
            "dcol": p["dcol"],
            "ea4": p["ea4"],
            "ea_col": p["ea_col"],
            "gmask": p["gmask"],
            "iota_row": iota_row,
            "inv_cnt": p["inv_cnt"],
            "p1_aug": np.concatenate(
                [np.asarray(inp["p1_w"], np.float32),
                 np.asarray(inp["p1_b"], np.float32)[None, :]], 0
            ),
            "ln_g4": np.broadcast_to(
                np.asarray(inp["ln_g"], np.float32)[None, :], (4, 128)
            ).copy(),
            "ln_b4": np.broadcast_to(
                np.asarray(inp["ln_b"], np.float32)[None, :], (4, 128)
            ).copy(),
            "p2_aug": np.concatenate(
                [np.asarray(inp["p2_w"], np.float32),
                 np.asarray(inp["p2_b"], np.float32)[None, :]], 0
            ),
        }
        for L in (1, 2):
            wl = np.asarray(inp[f"g{L}_wl"], np.float32)
            bl = np.asarray(inp[f"g{L}_bl"], np.float32)
            wr = np.asarray(inp[f"g{L}_wr"], np.float32)
            br = np.asarray(inp[f"g{L}_br"], np.float32)
            bias = np.asarray(inp[f"g{L}_bias"], np.float32)
            m[f"wl{L}_aug"] = np.concatenate([wl, bl[None, :]], 0).astype(f16)
            m[f"wr{L}_aug"] = np.concatenate([wr, br[None, :]], 0).astype(f16)
            m[f"bias{L}_row"] = np.broadcast_to(
                bias.reshape(1, HC), (128, HC)
            ).astype(f16).copy()
            m[f"att{L}_row"] = np.broadcast_to(
                np.asarray(inp[f"g{L}_att"], np.float32).reshape(1, HC), (128, HC)
            ).astype(f16).copy()
            m[f"we{L}_row"] = np.broadcast_to(
                np.asarray(inp[f"g{L}_we"], np.float32).reshape(1, HC), (128, HC)
            ).astype(f16).copy()
        in_maps.append(m)
    return in_maps


def kernel(**inputs):
    cores, packs, nspans, rows_per_core, rows_total, x_aug_T, node_row = _host_prep(
        inputs
    )
    key = (nspans, rows_total)
    if key not in _PROGRAM_CACHE:
        _PROGRAM_CACHE[key] = _build_program(nspans, rows_total)
    nc = _PROGRAM_CACHE[key]
    in_maps = _pack_inputs(
        inputs, cores, packs, nspans, rows_per_core, rows_total, x_aug_T
    )
    res = run_bass_kernel_spmd(nc, in_maps, core_ids=list(range(NCORES)))
    out = np.concatenate([res.results[k]["out"] for k in range(NCORES)], axis=0)
    return out.astype(np.float32)


if __name__ == "__main__":
    data = dict(np.load("/root/problem/inputs_cache.npz"))
    out = kernel(**data)
    exp = np.load("/root/problem/expected_np.npy")
    rel = np.linalg.norm(out - exp) / np.linalg.norm(exp)
    print("rel err:", rel)



# revision 3
# speedup vs baseline: 1.0359x; 1.0359x over previous
"""Trainium2 Bass kernel for nn_GATv2Base (gnn_message_passing).

Contract: kernel(**inputs) takes FULL unsharded inputs (same keys as
reference.setup_inputs()) and returns the FULL [32, 64] float32 output.

Sharding: 32 graphs -> 8 cores (4 graphs each, contiguous node ranges since
`batch` is sorted).  Edges (plus self-loops) are routed to the core owning
their dst node, sorted by dst, and packed into "spans" (<=127-node dst window,
2304 edge slots = 18 subgroups of 128 edges).  Node features live in a
span-major padded global layout so every per-span device address is static.
Layer 1 runs fully local (xl1 table computed replicated from x); between
layers one fp16 AllGather shares the xl2 table; the pooled per-graph MLP is
computed on the owning core.  Only the [4, 64] per-core outputs return to the
host.

All per-core inputs are packed into 5 device tensors (feat/wpack/meta_idx/
meta_val/mlp) to minimize per-call argument-marshalling overhead on the host
runtime.  _build_program(reps=N) unrolls the whole computation N times inside
one program; timing two variants isolates true device execution time from
dispatch overhead.
"""

import os
import sys

import numpy as np

for _p in ("/opt/trn_rl_repo", "/root/.axon_site/_ro/trn_rl_repo"):
    if os.path.isdir(_p) and _p not in sys.path:
        sys.path.insert(0, _p)

import concourse.bass as bass
import concourse.bacc as bacc
import concourse.mybir as mybir
import concourse.tile as tile
from concourse.bass_utils import run_bass_kernel_spmd

F32 = mybir.dt.float32
F16 = mybir.dt.float16
I16 = mybir.dt.int16
I32 = mybir.dt.int32
AF = mybir.ActivationFunctionType
ALU = mybir.AluOpType
AXX = mybir.AxisListType.X

N, E, H, C, NG = 50000, 800000, 4, 64, 32
HC = H * C
NCORES = 8
SLOT_H = 1152            # edge slots per table-half region (9 subgroups)
SPAN_EDGES = 2 * SLOT_H  # 2304 edge slots per span (18 subgroups of 128)
NSG = SPAN_EDGES // 128  # 18
NSG_H = SLOT_H // 128    # 9
SPAN_DST = 127           # dst window per span; slot 127 = pad marker

# meta_idx layout (i16 cols): src_idx_a 0:72, src_idx_b 72:144, xr_idx 144:288
MI_A0, MI_A1 = 0, SLOT_H // 16
MI_B0, MI_B1 = MI_A1, 2 * (SLOT_H // 16)
MI_X0, MI_X1 = MI_B1, MI_B1 + SPAN_EDGES // 16
# meta_val layout (f16 cols): dcol 0:18, eac 18:36, ea4 36:108, gmask 108:112
MV_D0, MV_D1 = 0, NSG
MV_E0, MV_E1 = NSG, 2 * NSG
MV_F0, MV_F1 = 2 * NSG, 6 * NSG
MV_G0, MV_G1 = 6 * NSG, 6 * NSG + 4
MV_W = MV_G1
# wpack rows (f16, width 256)
WP_WL1, WP_WR1 = 0, 65
WP_ATT1, WP_WE1, WP_BIAS1 = 130, 258, 386
WP_WL2, WP_WR2 = 514, 771
WP_ATT2, WP_WE2, WP_BIAS2 = 1028, 1156, 1284
WP_IOTA = 1412
WP_ROWS = 1540
# mlp rows (f32, width 128)
MP_P1, MP_LNG, MP_LNB, MP_P2, MP_CNT = 0, 257, 261, 265, 394
MP_ROWS = 398


# ----------------------------------------------------------------------------
# Host-side sharding / packing
# ----------------------------------------------------------------------------

def _host_prep(inp):
    x = np.asarray(inp["x"], dtype=np.float32)
    ei = np.asarray(inp["edge_index"], dtype=np.int32)
    ea_full = np.asarray(inp["edge_attr"], dtype=np.float32)[:, 0]
    batch = np.asarray(inp["batch"], dtype=np.int32)

    src0, dst0 = ei[0], ei[1]
    deg = np.maximum(np.bincount(dst0, minlength=N).astype(np.float64), 1.0)
    loop_attr = (
        np.bincount(dst0, weights=ea_full.astype(np.float64), minlength=N) / deg
    ).astype(np.float32)
    src = np.concatenate([src0, np.arange(N, dtype=np.int32)])
    dst = np.concatenate([dst0, np.arange(N, dtype=np.int32)])
    eattr = np.concatenate([ea_full, loop_attr]).astype(np.float32)

    gcounts = np.bincount(batch, minlength=NG)
    gstart = np.concatenate([[0], np.cumsum(gcounts)])
    core_n0 = np.array([gstart[4 * k] for k in range(NCORES)] + [N], dtype=np.int64)

    order = np.argsort(dst, kind="stable")
    src, dst, eattr = src[order], dst[order], eattr[order]
    edge_lo = np.searchsorted(dst, core_n0[:-1], "left")
    edge_hi = np.searchsorted(dst, core_n0[1:], "left")

    # src owner core (cores 0-3 -> table half A, 4-7 -> half B); stable
    # under span-count changes so it can drive packing.
    src_owner = np.searchsorted(core_n0[1:], src, "right")
    src_in_a = src_owner < (NCORES // 2)

    cores = []
    for k in range(NCORES):
        n0, n1 = int(core_n0[k]), int(core_n0[k + 1])
        s, e = int(edge_lo[k]), int(edge_hi[k])
        cd = dst[s:e]
        ca = src_in_a[s:e]
        nlocal = n1 - n0
        node_edge_start = np.searchsorted(cd, n0 + np.arange(nlocal + 1))
        cumA = np.concatenate([[0], np.cumsum(ca)])  # over edges
        spans = []
        b = 0
        while b < nlocal:
            bend = b
            while bend < nlocal and (bend - b) < SPAN_DST:
                e0, e1 = node_edge_start[b], node_edge_start[bend + 1]
                nA = cumA[e1] - cumA[e0]
                nB = (e1 - e0) - nA
                if nA > SLOT_H or nB > SLOT_H:
                    break
                bend += 1
            assert bend > b, "single node exceeds span edge capacity"
            spans.append(
                (b, bend - b, int(node_edge_start[b]), int(node_edge_start[bend]))
            )
            b = bend
        cores.append(
            dict(n0=n0, n1=n1, spans=spans, src=src[s:e], dst=cd, ea=eattr[s:e],
                 in_a=ca)
        )

    nspans = max(len(c["spans"]) for c in cores)
    rows_per_core = nspans * 128
    rows_total = NCORES * rows_per_core

    # global padded row per node
    node_row = np.zeros(N, dtype=np.int64)
    for k, c in enumerate(cores):
        for si, (b, nb, _, _) in enumerate(c["spans"]):
            nodes = np.arange(c["n0"] + b, c["n0"] + b + nb)
            node_row[nodes] = k * rows_per_core + si * 128 + (nodes - c["n0"] - b)

    # x padded, transposed, with ones row (for encoder rhs)
    x_pad = np.zeros((rows_total, 4), dtype=np.float32)
    x_pad[node_row] = x
    x_aug_T = np.concatenate(
        [x_pad.T, np.ones((1, rows_total), dtype=np.float32)], axis=0
    )  # [5, R]

    packs = []
    half_rows = rows_total // 2
    assert half_rows <= 32767, f"table half {half_rows} exceeds int16 index range"

    def wrap_idx16(vals):
        # vals: [SLOT] int -> wrapped [128, SLOT//16] int16 (16-part wrap,
        # replicated over the 8 q7 core groups)
        slot = len(vals)
        base = np.zeros((16, slot // 16), dtype=np.int16)
        i = np.arange(slot)
        base[i % 16, i // 16] = vals.astype(np.int16)
        return np.tile(base, (8, 1))

    for k, c in enumerate(cores):
        meta_idx = np.zeros((nspans, 128, MI_X1), dtype=np.int16)
        meta_val = np.zeros((nspans, 128, MV_W), dtype=np.float16)
        meta_val[:, :, MV_D0:MV_D1] = np.float16(127.0)
        for si, (b, nb, e0, e1) in enumerate(c["spans"]):
            ina = c["in_a"][e0:e1]
            esrc = node_row[c["src"][e0:e1]]
            edrel = (c["dst"][e0:e1] - c["n0"] - b).astype(np.int64)
            eea = c["ea"][e0:e1]
            # slots: A edges first (in region [0, SLOT_H)), then B edges at
            # [SLOT_H, 2*SLOT_H); pads keep idx 0 / drel 127 / ea 0
            ia = np.where(ina)[0]
            ib = np.where(~ina)[0]
            slots = np.empty(len(ina), dtype=np.int64)
            slots[ia] = np.arange(len(ia))
            slots[ib] = SLOT_H + np.arange(len(ib))
            av = np.zeros(SLOT_H, dtype=np.int64)
            av[:len(ia)] = esrc[ia]
            bv = np.zeros(SLOT_H, dtype=np.int64)
            bv[:len(ib)] = esrc[ib] - half_rows
            meta_idx[si, :, MI_A0:MI_A1] = wrap_idx16(av)
            meta_idx[si, :, MI_B0:MI_B1] = wrap_idx16(bv)
            xv = np.full(SPAN_EDGES, si * 128 + 127, dtype=np.int64)
            xv[slots] = si * 128 + edrel
            meta_idx[si, :, MI_X0:MI_X1] = wrap_idx16(xv)
            p, sg = slots % 128, slots // 128
            meta_val[si, p, MV_D0 + sg] = edrel.astype(np.float16)
            meta_val[si, p, MV_E0 + sg] = eea.astype(np.float16)
            for hh in range(4):
                meta_val[si, p, MV_F0 + sg * 4 + hh] = eea.astype(np.float16)
            nodes = np.arange(c["n0"] + b, c["n0"] + b + nb)
            gl = batch[nodes] - 4 * k
            meta_val[si, np.arange(nb), MV_G0 + gl] = np.float16(1.0)
        inv_cnt = np.zeros((4,), dtype=np.float32)
        for gg in range(4):
            cnt = max(int(gcounts[4 * k + gg]), 1)
            inv_cnt[gg] = 1.0 / cnt
        packs.append(
            dict(
                meta_idx=meta_idx,
                meta_val=meta_val,
                inv_cnt=inv_cnt,
                own_cols=np.arange(
                    k * rows_per_core, (k + 1) * rows_per_core, dtype=np.int64
                ),
            )
        )
    return cores, packs, nspans, rows_per_core, rows_total, x_aug_T, node_row


# ----------------------------------------------------------------------------
# Device program
# ----------------------------------------------------------------------------

_PROGRAM_CACHE = {}


def _build_program(nspans, rows_total, reps=1):
    rows_per_core = nspans * 128
    nblocks = rows_total // 128

    nc = bacc.Bacc()
    tcx = tile.TileContext(nc)

    t_feat = nc.dram_tensor(
        "feat", [5, rows_total + rows_per_core + 64], F32, kind="ExternalInput"
    )
    t_wpack = nc.dram_tensor("wpack", [WP_ROWS, HC], F16, kind="ExternalInput")
    t_midx = nc.dram_tensor(
        "meta_idx", [nspans, 128, MI_X1], I16, kind="ExternalInput"
    )
    t_mval = nc.dram_tensor(
        "meta_val", [nspans, 128, MV_W], F16, kind="ExternalInput"
    )
    t_mlp = nc.dram_tensor("mlp", [MP_ROWS, 128], F32, kind="ExternalInput")
    t_out = nc.dram_tensor("out", [4, 64], F32, kind="ExternalOutput")

    # ---- internal DRAM ----
    t_xl1 = nc.dram_tensor("xl1_tbl", [rows_total, HC], F16)
    t_xr1 = nc.dram_tensor("xr1_own", [rows_per_core, HC], F16)
    t_h1 = nc.dram_tensor("h1_own", [rows_per_core, HC], F16)
    t_xr2 = nc.dram_tensor("xr2_own", [rows_per_core, HC], F16)
    t_xl2_in = nc.dram_tensor("xl2_own_cc", [rows_per_core, HC], F16)
    t_xl2 = nc.dram_tensor("xl2_tbl", [rows_total, HC], F16, addr_space="Shared")

    from contextlib import ExitStack
    with tcx as tc, ExitStack() as es:
        # ------------------------------------------------------------------
        # constants in SBUF (loaded once, reused by every rep)
        # ------------------------------------------------------------------
        cpool = es.enter_context(tc.tile_pool(name="consts", bufs=1))
        enc_aug = cpool.tile([5, 64], F32)
        nc.sync.dma_start(
            out=enc_aug[:],
            in_=t_feat[:, rows_total + rows_per_core:rows_total + rows_per_core + 64],
        )
        iota_rep = cpool.tile([128, 128], F16)
        nc.sync.dma_start(out=iota_rep[:], in_=t_wpack[WP_IOTA:WP_IOTA + 128, 0:128])
        reps_t = {}
        for L, (r_att, r_we, r_bias) in (
            (1, (WP_ATT1, WP_WE1, WP_BIAS1)),
            (2, (WP_ATT2, WP_WE2, WP_BIAS2)),
        ):
            for nm, r0 in (("att_row", r_att), ("we_row", r_we), ("bias_row", r_bias)):
                rep = cpool.tile([128, HC], F16, tag=f"rep{L}{nm}")
                nc.sync.dma_start(out=rep[:], in_=t_wpack[r0:r0 + 128, :])
                reps_t[(L, nm)] = rep
        ones_col = cpool.tile([1, 128], F16)
        nc.vector.memset(ones_col[:], 1.0)

        wpool = es.enter_context(tc.tile_pool(name="weights", bufs=1))
        wl1 = wpool.tile([65, HC], F16)
        wr1 = wpool.tile([65, HC], F16)
        nc.sync.dma_start(out=wl1[:], in_=t_wpack[WP_WL1:WP_WL1 + 65, :])
        nc.sync.dma_start(out=wr1[:], in_=t_wpack[WP_WR1:WP_WR1 + 65, :])
        w2_tiles = {}
        for nm, r0 in (("wl_aug", WP_WL2), ("wr_aug", WP_WR2)):
            a = wpool.tile([128, HC], F16, tag=f"{nm}a")
            b = wpool.tile([128, HC], F16, tag=f"{nm}b")
            cbias = wpool.tile([1, HC], F16, tag=f"{nm}c")
            nc.sync.dma_start(out=a[:], in_=t_wpack[r0:r0 + 128, :])
            nc.sync.dma_start(out=b[:], in_=t_wpack[r0 + 128:r0 + 256, :])
            nc.sync.dma_start(out=cbias[:], in_=t_wpack[r0 + 256:r0 + 257, :])
            w2_tiles[nm] = (a, b, cbias)
        # MLP constants
        mpool = es.enter_context(tc.tile_pool(name="mlpc", bufs=1))
        p1a = mpool.tile([128, 128], F32)
        p1b = mpool.tile([128, 128], F32)
        p1c = mpool.tile([1, 128], F32)
        nc.sync.dma_start(out=p1a[:], in_=t_mlp[MP_P1:MP_P1 + 128, :])
        nc.sync.dma_start(out=p1b[:], in_=t_mlp[MP_P1 + 128:MP_P1 + 256, :])
        nc.sync.dma_start(out=p1c[:], in_=t_mlp[MP_P1 + 256:MP_P1 + 257, :])
        p2a = mpool.tile([128, 64], F32)
        p2c = mpool.tile([1, 64], F32)
        nc.sync.dma_start(out=p2a[:], in_=t_mlp[MP_P2:MP_P2 + 128, 0:64])
        nc.sync.dma_start(out=p2c[:], in_=t_mlp[MP_P2 + 128:MP_P2 + 129, 0:64])
        lng = mpool.tile([4, 128], F32)
        nc.sync.dma_start(out=lng[:], in_=t_mlp[MP_LNG:MP_LNG + 4, :])
        lnb = mpool.tile([4, 128], F32)
        nc.sync.dma_start(out=lnb[:], in_=t_mlp[MP_LNB:MP_LNB + 4, :])
        icnt = mpool.tile([4, 1], F32)
        nc.sync.dma_start(out=icnt[:], in_=t_mlp[MP_CNT:MP_CNT + 4, 0:1])
        ident = mpool.tile([128, 128], F32)
        from concourse.masks import make_identity
        make_identity(nc, ident[:])
        onesg = mpool.tile([1, 4], F32)
        nc.vector.memset(onesg[:], 1.0)

        def encode_block(pool, ppool, xaugT_ap):
            """xaugT_ap: [5, 128] f32 dram slice -> h0T_aug [65, 128] f16 tile"""
            xT = pool.tile([5, 128], F32, tag="xT")
            nc.sync.dma_start(out=xT[:], in_=xaugT_ap)
            h0psum = ppool.tile([64, 128], F32, tag="h0ps")
            nc.tensor.matmul(out=h0psum[:], lhsT=enc_aug[:], rhs=xT[:],
                             start=True, stop=True)
            h0T = pool.tile([65, 128], F16, tag="h0T")
            nc.scalar.activation(out=h0T[0:64, :], in_=h0psum[:], func=AF.Relu)
            nc.vector.tensor_copy(out=h0T[64:65, :], in_=ones_col[:])
            return h0T

        def build_rep():
            # --------------------------------------------------------------
            # Phase 1: encoder + xl1 for ALL rows
            # --------------------------------------------------------------
            with tc.tile_pool(name="p1", bufs=3) as pool, \
                 tc.tile_pool(name="p1ps", bufs=2, space="PSUM") as ppool:
                for blk in range(nblocks):
                    h0T = encode_block(pool, ppool,
                                       t_feat[:, blk * 128:(blk + 1) * 128])
                    xlp = ppool.tile([128, HC], F32, tag="xlps")
                    nc.tensor.matmul(out=xlp[:], lhsT=h0T[:], rhs=wl1[:],
                                     start=True, stop=True)
                    xls = pool.tile([128, HC], F16, tag="xls")
                    if blk % 2 == 0:
                        nc.vector.tensor_copy(out=xls[:], in_=xlp[:])
                    else:
                        nc.scalar.copy(out=xls[:], in_=xlp[:])
                    nc.sync.dma_start(
                        out=t_xl1[blk * 128:(blk + 1) * 128, :], in_=xls[:]
                    )

                # own xr1
                for s in range(nspans):
                    h0T = encode_block(
                        pool, ppool,
                        t_feat[:, rows_total + s * 128:rows_total + (s + 1) * 128],
                    )
                    xrp = ppool.tile([128, HC], F32, tag="xlps")
                    nc.tensor.matmul(out=xrp[:], lhsT=h0T[:], rhs=wr1[:],
                                     start=True, stop=True)
                    xrs = pool.tile([128, HC], F16, tag="xls")
                    nc.vector.tensor_copy(out=xrs[:], in_=xrp[:])
                    nc.sync.dma_start(
                        out=t_xr1[s * 128:(s + 1) * 128, :], in_=xrs[:]
                    )

            # --------------------------------------------------------------
            # GAT span loop (shared for both layers)
            # --------------------------------------------------------------
            def gat_layer(L, xl_tbl, xr_tbl, h_sink):
                """h_sink(s, htile, mval): consume flush output [128, HC] f16."""
                att_rep = reps_t[(L, "att_row")]
                we_rep = reps_t[(L, "we_row")]
                bias_rep = reps_t[(L, "bias_row")]
                with tc.tile_pool(name=f"g{L}", bufs=2) as pool, \
                     tc.tile_pool(name=f"g{L}b", bufs=3) as spool, \
                     tc.tile_pool(name=f"g{L}ps", bufs=2, space="PSUM") as ppool:
                    half_rows = rows_total // 2
                    for s in range(nspans):
                        midx = spool.tile([128, MI_X1], I16, tag="midx")
                        nc.sync.dma_start(out=midx[:], in_=t_midx[s, :, :])
                        mval = spool.tile([128, MV_W], F16, tag="mval")
                        nc.sync.dma_start(out=mval[:], in_=t_mval[s, :, :])
                        dcol = spool.tile([128, NSG], F32, tag="dcolF")
                        nc.vector.tensor_copy(out=dcol[:],
                                              in_=mval[:, MV_D0:MV_D1])
                        eac = mval[:, MV_E0:MV_E1]
                        ea4 = mval[:, MV_F0:MV_F1]
                        xr_fl = spool.tile([128, HC], F16, tag="xrfl")
                        nc.sync.dma_start(
                            out=xr_fl[:], in_=xr_tbl[s * 128:(s + 1) * 128, :]
                        )
                        xrb = spool.tile([128, HC], F16, tag="xrb")
                        nc.vector.tensor_tensor(out=xrb[:], in0=xr_fl[:],
                                                in1=bias_rep[:], op=ALU.subtract)

                        # G = xl[src] (two half-table gathers), R = xr[dst],
                        # v = (we*ea + R) + G
                        G = pool.tile([128, NSG, HC], F16, tag="G")
                        nc.gpsimd.dma_gather(
                            G[:, 0:NSG_H, :], xl_tbl[0:half_rows, :],
                            midx[:, MI_A0:MI_A1],
                            SLOT_H, SLOT_H, HC, single_packet=False,
                        )
                        nc.gpsimd.dma_gather(
                            G[:, NSG_H:NSG, :], xl_tbl[half_rows:, :],
                            midx[:, MI_B0:MI_B1],
                            SLOT_H, SLOT_H, HC, single_packet=False,
                        )
                        R = pool.tile([128, NSG, HC], F16, tag="R")
                        nc.gpsimd.dma_gather(
                            R[:, :, :], xr_tbl[:, :], midx[:, MI_X0:MI_X1],
                            SPAN_EDGES, SPAN_EDGES, HC, single_packet=False,
                        )
                        v = pool.tile([128, NSG, HC], F16, tag="v")
                        for sg in range(NSG):
                            nc.vector.scalar_tensor_tensor(
                                out=v[:, sg, :], in0=we_rep[:],
                                scalar=eac[:, sg:sg + 1], in1=R[:, sg, :],
                                op0=ALU.mult, op1=ALU.add,
                            )
                        nc.vector.tensor_tensor(
                            out=v[:, :, :], in0=v[:, :, :], in1=G[:, :, :],
                            op=ALU.add
                        )

                        # u = lrelu(v), z = u*att, alpha = per-head sum
                        u = pool.tile([128, NSG, HC], F16, tag="u")
                        nc.scalar.activation(out=u[:, :, :], in_=v[:, :, :],
                                             func=AF.Lrelu, alpha=0.2)
                        z = pool.tile([128, NSG, HC], F16, tag="z")
                        nc.vector.tensor_tensor(
                            out=z[:, :, :], in0=u[:, :, :],
                            in1=att_rep[:].rearrange(
                                "p (o c) -> p o c", o=1
                            ).broadcast_to((128, NSG, HC)), op=ALU.mult
                        )
                        # per-head sums via binary fold tree
                        zf = pool.tile([128, NSG, 4, 32], F16, tag="zf")
                        z4 = z[:].rearrange("p s (h c) -> p s h c", h=4)
                        nc.vector.tensor_tensor(
                            out=zf[:, :, :, :], in0=z4[:, :, :, 0:32],
                            in1=z4[:, :, :, 32:64], op=ALU.add,
                        )
                        w = 16
                        while w >= 2:
                            nc.vector.tensor_tensor(
                                out=zf[:, :, :, 0:w], in0=zf[:, :, :, 0:w],
                                in1=zf[:, :, :, w:2 * w], op=ALU.add,
                            )
                            w //= 2
                        alpha = spool.tile([128, 4 * NSG], F32, tag="alpha")
                        nc.vector.tensor_tensor(
                            out=alpha[:].rearrange("p (s h o) -> p s h o",
                                                   h=4, o=1),
                            in0=zf[:, :, :, 0:1], in1=zf[:, :, :, 1:2],
                            op=ALU.add,
                        )
                        exF = spool.tile([128, 4 * NSG], F32, tag="exF")
                        nc.scalar.activation(out=exF[:], in_=alpha[:], func=AF.Exp)
                        ex = spool.tile([128, 4 * NSG], F16, tag="ex")
                        nc.vector.tensor_copy(out=ex[:], in_=exF[:])
                        exea = spool.tile([128, 4 * NSG], F16, tag="exea")
                        nc.vector.tensor_tensor(out=exea[:], in0=ex[:], in1=ea4[:],
                                                op=ALU.mult)

                        # m2 = ex * v (per head), S one-hot, agg matmuls
                        m2 = pool.tile([128, NSG, HC], F16, tag="m2")
                        S = pool.tile([128, NSG, 128], F16, tag="S")
                        accM = ppool.tile([128, HC], F32, tag="accM")
                        accE = ppool.tile([128, 4], F32, tag="accE")
                        accX = ppool.tile([128, 4], F32, tag="accX")
                        for sg in range(NSG):
                            for hh in range(4):
                                nc.vector.tensor_scalar(
                                    out=m2[:, sg, hh * C:(hh + 1) * C],
                                    in0=v[:, sg, hh * C:(hh + 1) * C],
                                    scalar1=exF[:, sg * 4 + hh:sg * 4 + hh + 1],
                                    scalar2=None, op0=ALU.mult,
                                )
                            nc.vector.tensor_scalar(
                                out=S[:, sg, :], in0=iota_rep[:],
                                scalar1=dcol[:, sg:sg + 1], scalar2=None,
                                op0=ALU.is_equal,
                            )
                            nc.tensor.matmul(out=accM[:], lhsT=S[:, sg, :],
                                             rhs=m2[:, sg, :], start=(sg == 0),
                                             stop=(sg == NSG - 1))
                            nc.tensor.matmul(out=accE[:], lhsT=S[:, sg, :],
                                             rhs=ex[:, sg * 4:sg * 4 + 4],
                                             start=(sg == 0), stop=(sg == NSG - 1))
                            nc.tensor.matmul(out=accX[:], lhsT=S[:, sg, :],
                                             rhs=exea[:, sg * 4:sg * 4 + 4],
                                             start=(sg == 0), stop=(sg == NSG - 1))

                        # flush: h = relu(acc/den - xr' - we*(eaden/den))
                        A = spool.tile([128, HC], F32, tag="A")
                        nc.scalar.copy(out=A[:], in_=accM[:])
                        den = spool.tile([128, 4], F32, tag="den")
                        nc.vector.tensor_scalar(
                            out=den[:], in0=accE[:], scalar1=1e-30,
                            scalar2=None, op0=ALU.add,
                        )
                        rden = spool.tile([128, 4], F32, tag="rden")
                        nc.vector.reciprocal(out=rden[:], in_=den[:])
                        eaden = spool.tile([128, 4], F32, tag="eaden")
                        nc.vector.tensor_copy(out=eaden[:], in_=accX[:])
                        eard_n = spool.tile([128, 4], F32, tag="eardn")
                        for hh in range(4):
                            nc.vector.tensor_scalar(
                                out=eard_n[:, hh:hh + 1],
                                in0=eaden[:, hh:hh + 1],
                                scalar1=rden[:, hh:hh + 1], scalar2=-1.0,
                                op0=ALU.mult, op1=ALU.mult,
                            )
                        hT = spool.tile([128, HC], F16, tag="hT")
                        for hh in range(4):
                            blks = slice(hh * C, (hh + 1) * C)
                            nc.vector.scalar_tensor_tensor(
                                out=hT[:, blks], in0=A[:, blks],
                                scalar=rden[:, hh:hh + 1], in1=xrb[:, blks],
                                op0=ALU.mult, op1=ALU.subtract,
                            )
                            nc.vector.scalar_tensor_tensor(
                                out=hT[:, blks], in0=we_rep[:, blks],
                                scalar=eard_n[:, hh:hh + 1], in1=hT[:, blks],
                                op0=ALU.mult, op1=ALU.add,
                            )
                        hOut = spool.tile([128, HC], F16, tag="hOut")
                        nc.scalar.activation(out=hOut[:], in_=hT[:], func=AF.Relu)
                        h_sink(s, hOut, mval, pool, ppool)

            # layer 1: sink writes h1 to DRAM
            def h1_sink(s, hOut, mval, pool, ppool):
                nc.sync.dma_start(out=t_h1[s * 128:(s + 1) * 128, :], in_=hOut[:])

            gat_layer(1, t_xl1, t_xr1, h1_sink)

            # --------------------------------------------------------------
            # Phase 4: xl2/xr2 from h1 (own spans)
            # --------------------------------------------------------------
            with tc.tile_pool(name="p4", bufs=3) as pool, \
                 tc.tile_pool(name="p4ps", bufs=2, space="PSUM") as ppool:
                for s in range(nspans):
                    h1T0 = pool.tile([128, 128], F16, tag="h1T0")
                    h1T1 = pool.tile([128, 128], F16, tag="h1T1")
                    nc.sync.dma_start(
                        out=h1T0[:], in_=t_h1[s * 128:(s + 1) * 128, 0:128],
                        transpose=True,
                    )
                    nc.sync.dma_start(
                        out=h1T1[:], in_=t_h1[s * 128:(s + 1) * 128, 128:256],
                        transpose=True,
                    )
                    for nm, sink in (("wl_aug", t_xl2_in), ("wr_aug", t_xr2)):
                        wa, wb, wc = w2_tiles[nm]
                        ps = ppool.tile([128, HC], F32, tag="ps")
                        nc.tensor.matmul(out=ps[:], lhsT=h1T0[:], rhs=wa[:],
                                         start=True, stop=False)
                        nc.tensor.matmul(out=ps[:], lhsT=h1T1[:], rhs=wb[:],
                                         start=False, stop=False)
                        nc.tensor.matmul(out=ps[:], lhsT=ones_col[:],
                                         rhs=wc[:], start=False, stop=True)
                        xs = pool.tile([128, HC], F16, tag="xs")
                        nc.vector.tensor_copy(out=xs[:], in_=ps[:])
                        nc.sync.dma_start(out=sink[s * 128:(s + 1) * 128, :],
                                          in_=xs[:])

            # --------------------------------------------------------------
            # Phase 5: AllGather xl2
            # --------------------------------------------------------------
            nc.gpsimd.collective_compute(
                "AllGather",
                ALU.bypass,
                replica_groups=[list(range(NCORES))],
                ins=[t_xl2_in.ap().opt()],
                outs=[t_xl2.ap().opt()],
            )

            # --------------------------------------------------------------
            # Phase 6: GAT layer 2 with fused pooling
            # --------------------------------------------------------------
            with tc.tile_pool(name="gpool_ps", bufs=1, space="PSUM") as gpool_ps:
                gpsum = gpool_ps.tile([4, HC], F32)

                def h2_sink(s, hOut, mval, pool, ppool):
                    nc.tensor.matmul(out=gpsum[:], lhsT=mval[:, MV_G0:MV_G1],
                                     rhs=hOut[:],
                                     start=(s == 0), stop=(s == nspans - 1))

                gat_layer(2, t_xl2, t_xr2, h2_sink)

                # ----------------------------------------------------------
                # Phase 7: pooling -> MLP -> out
                # ----------------------------------------------------------
                with tc.tile_pool(name="mlp", bufs=1) as pool, \
                     tc.tile_pool(name="mlp_ps", bufs=2, space="PSUM") as ppool:
                    g = pool.tile([4, HC], F32)
                    nc.vector.tensor_scalar(out=g[:], in0=gpsum[:],
                                            scalar1=icnt[:, 0:1],
                                            scalar2=None, op0=ALU.mult)
                    # gT via PE transpose (two halves)
                    gT = pool.tile([128, 8], F32)
                    for half in range(2):
                        tp = ppool.tile([128, 128], F32, tag="tp")
                        nc.tensor.transpose(
                            out=tp[:, 0:4],
                            in_=g[:, half * 128:(half + 1) * 128],
                            identity=ident[0:4, 0:4],
                        )
                        nc.vector.tensor_copy(out=gT[:, half * 4:half * 4 + 4],
                                              in_=tp[:, 0:4])
                    z1p = ppool.tile([4, 128], F32, tag="z1p")
                    nc.tensor.matmul(out=z1p[:], lhsT=gT[:, 0:4], rhs=p1a[:],
                                     start=True, stop=False)
                    nc.tensor.matmul(out=z1p[:], lhsT=gT[:, 4:8], rhs=p1b[:],
                                     start=False, stop=False)
                    nc.tensor.matmul(out=z1p[:], lhsT=onesg[:], rhs=p1c[:],
                                     start=False, stop=True)
                    z1 = pool.tile([4, 128], F32)
                    nc.vector.tensor_copy(out=z1[:], in_=z1p[:])
                    # layernorm over free dim (128)
                    mu = pool.tile([4, 1], F32)
                    nc.vector.reduce_sum(out=mu[:], in_=z1[:], axis=AXX)
                    nc.vector.tensor_scalar(out=mu[:], in0=mu[:],
                                            scalar1=1.0 / 128,
                                            scalar2=None, op0=ALU.mult)
                    zc = pool.tile([4, 128], F32)
                    nc.vector.tensor_scalar(out=zc[:], in0=z1[:],
                                            scalar1=mu[:, 0:1],
                                            scalar2=None, op0=ALU.subtract)
                    sq = pool.tile([4, 128], F32)
                    nc.vector.tensor_tensor(out=sq[:], in0=zc[:], in1=zc[:],
                                            op=ALU.mult)
                    var = pool.tile([4, 1], F32)
                    nc.vector.reduce_sum(out=var[:], in_=sq[:], axis=AXX)
                    nc.vector.tensor_scalar(out=var[:], in0=var[:],
                                            scalar1=1.0 / 128,
                                            scalar2=1e-5, op0=ALU.mult,
                                            op1=ALU.add)
                    std = pool.tile([4, 1], F32)
                    nc.scalar.activation(out=std[:], in_=var[:], func=AF.Sqrt)
                    rstd = pool.tile([4, 1], F32)
                    nc.vector.reciprocal(out=rstd[:], in_=std[:])
                    zn = pool.tile([4, 128], F32)
                    nc.vector.tensor_scalar(out=zn[:], in0=zc[:],
                                            scalar1=rstd[:, 0:1],
                                            scalar2=None, op0=ALU.mult)
                    nc.vector.tensor_tensor(out=zn[:], in0=zn[:], in1=lng[:],
                                            op=ALU.mult)
                    nc.vector.tensor_tensor(out=zn[:], in0=zn[:], in1=lnb[:],
                                            op=ALU.add)
                    nc.scalar.activation(out=zn[:], in_=zn[:], func=AF.Relu)
                    # z2 = relu(zn @ p2 + b2)
                    znT = pool.tile([128, 4], F32)
                    tp2 = ppool.tile([128, 128], F32, tag="tp")
                    nc.tensor.transpose(out=tp2[:, 0:4], in_=zn[:],
                                        identity=ident[0:4, 0:4])
                    nc.vector.tensor_copy(out=znT[:], in_=tp2[:, 0:4])
                    z2p = ppool.tile([4, 64], F32, tag="z2p")
                    nc.tensor.matmul(out=z2p[:], lhsT=znT[:], rhs=p2a[:],
                                     start=True, stop=False)
                    nc.tensor.matmul(out=z2p[:], lhsT=onesg[:], rhs=p2c[:],
                                     start=False, stop=True)
                    zout = pool.tile([4, 64], F32)
                    nc.scalar.activation(out=zout[:], in_=z2p[:], func=AF.Relu)
                    nc.sync.dma_start(out=t_out[:], in_=zout[:])

        for _rep in range(reps):
            build_rep()

    nc.finalize()
    return nc


# ----------------------------------------------------------------------------
# Entry point
# ----------------------------------------------------------------------------

def _pack_inputs(inp, cores, packs, nspans, rows_per_core, rows_total, x_aug_T):
    f16 = np.float16
    f32 = np.float32
    # shared (replicated) blocks
    wpack = np.zeros((WP_ROWS, HC), dtype=f16)

    def aug(w, b):
        return np.concatenate(
            [np.asarray(w, f32), np.asarray(b, f32)[None, :]], 0
        ).astype(f16)

    wpack[WP_WL1:WP_WL1 + 65] = aug(inp["g1_wl"], inp["g1_bl"])
    wpack[WP_WR1:WP_WR1 + 65] = aug(inp["g1_wr"], inp["g1_br"])
    wpack[WP_WL2:WP_WL2 + 257] = aug(inp["g2_wl"], inp["g2_bl"])
    wpack[WP_WR2:WP_WR2 + 257] = aug(inp["g2_wr"], inp["g2_br"])
    for L, (r_att, r_we, r_bias) in (
        (1, (WP_ATT1, WP_WE1, WP_BIAS1)),
        (2, (WP_ATT2, WP_WE2, WP_BIAS2)),
    ):
        wpack[r_att:r_att + 128] = np.broadcast_to(
            np.asarray(inp[f"g{L}_att"], f32).reshape(1, HC), (128, HC)
        ).astype(f16)
        wpack[r_we:r_we + 128] = np.broadcast_to(
            np.asarray(inp[f"g{L}_we"], f32).reshape(1, HC), (128, HC)
        ).astype(f16)
        wpack[r_bias:r_bias + 128] = np.broadcast_to(
            np.asarray(inp[f"g{L}_bias"], f32).reshape(1, HC), (128, HC)
        ).astype(f16)
    wpack[WP_IOTA:WP_IOTA + 128, 0:128] = np.broadcast_to(
        np.arange(128, dtype=f16)[None, :], (128, 128)
    )

    mlp = np.zeros((MP_ROWS, 128), dtype=f32)
    mlp[MP_P1:MP_P1 + 257] = np.concatenate(
        [np.asarray(inp["p1_w"], f32), np.asarray(inp["p1_b"], f32)[None, :]], 0
    )
    mlp[MP_LNG:MP_LNG + 4] = np.asarray(inp["ln_g"], f32)[None, :]
    mlp[MP_LNB:MP_LNB + 4] = np.asarray(inp["ln_b"], f32)[None, :]
    mlp[MP_P2:MP_P2 + 129, 0:64] = np.concatenate(
        [np.asarray(inp["p2_w"], f32), np.asarray(inp["p2_b"], f32)[None, :]], 0
    )

    enc_aug = np.concatenate(
        [np.asarray(inp["enc_w"], f32), np.asarray(inp["enc_b"], f32)[None, :]], 0
    )  # [5, 64]

    in_maps = []
    for k in range(NCORES):
        p = packs[k]
        feat = np.zeros((5, rows_total + rows_per_core + 64), dtype=f32)
        feat[:, 0:rows_total] = x_aug_T
        feat[:, rows_total:rows_total + rows_per_core] = x_aug_T[:, p["own_cols"]]
        feat[:, rows_total + rows_per_core:] = enc_aug
        mlp_k = mlp.copy()
        mlp_k[MP_CNT:MP_CNT + 4, 0] = p["inv_cnt"]
        in_maps.append({
            "feat": feat,
            "wpack": wpack,
            "meta_idx": p["meta_idx"],
            "meta_val": p["meta_val"].view(np.float16),
            "mlp": mlp_k,
        })
    return in_maps


def kernel(**inputs):
    cores, packs, nspans, rows_per_core, rows_total, x_aug_T, node_row = _host_prep(
        inputs
    )
    key = (nspans, rows_total)
    if key not in _PROGRAM_CACHE:
        _PROGRAM_CACHE[key] = _build_program(nspans, rows_total)
    nc = _PROGRAM_CACHE[key]
    in_maps = _pack_inputs(
        inputs, cores, packs, nspans, rows_per_core, rows_total, x_aug_T
    )
    res = run_bass_kernel_spmd(nc, in_maps, core_ids=list(range(NCORES)))
    out = np.concatenate([res.results[k]["out"] for k in range(NCORES)], axis=0)
    return out.astype(np.float32)


if __name__ == "__main__":
    data = dict(np.load("/root/problem/inputs_cache.npz"))
    out = kernel(**data)
    exp = np.load("/root/problem/expected_np.npy")
    rel = np.linalg.norm(out - exp) / np.linalg.norm(exp)
    print("rel err:", rel)


# revision 7
# speedup vs baseline: 13.5647x; 13.0947x over previous
"""Trainium2 Bass kernel for nn_GATv2Base (gnn_message_passing).

Contract: kernel(**inputs) takes FULL unsharded inputs (same keys as
reference.setup_inputs()) and returns the FULL [32, 64] float32 output.

Sharding: 32 graphs -> 8 cores (4 graphs each, contiguous node ranges since
`batch` is sorted).  Edges (plus self-loops) are routed to the core owning
their dst node, sorted by dst, and packed into "spans" (<=127-node dst window,
2304 edge slots = 18 subgroups of 128 edges).  Node features live in a
span-major padded global layout so every per-span device address is static.
Layer 1 runs fully local (xl1 table computed replicated from x); between
layers one fp16 AllGather shares the xl2 table; the pooled per-graph MLP is
computed on the owning core.  Only the [4, 64] per-core outputs return to the
host.

All per-core inputs are packed into 5 device tensors (feat/wpack/meta_idx/
meta_val/mlp) to minimize per-call argument-marshalling overhead on the host
runtime.  _build_program(reps=N) unrolls the whole computation N times inside
one program; timing two variants isolates true device execution time from
dispatch overhead.
"""

import os
import sys

import numpy as np

for _p in ("/opt/trn_rl_repo", "/root/.axon_site/_ro/trn_rl_repo"):
    if os.path.isdir(_p) and _p not in sys.path:
        sys.path.insert(0, _p)

import concourse.bass as bass
import concourse.bacc as bacc
import concourse.mybir as mybir
import concourse.tile as tile
from concourse.bass_utils import run_bass_kernel_spmd

F32 = mybir.dt.float32
F16 = mybir.dt.float16
I16 = mybir.dt.int16
I32 = mybir.dt.int32
AF = mybir.ActivationFunctionType
ALU = mybir.AluOpType
AXX = mybir.AxisListType.X

N, E, H, C, NG = 50000, 800000, 4, 64, 32
HC = H * C
NCORES = 8
SLOT_H = 1152            # edge slots per table-half region (9 subgroups)
SPAN_EDGES = 2 * SLOT_H  # 2304 edge slots per span (18 subgroups of 128)
NSG = SPAN_EDGES // 128  # 18
NSG_H = SLOT_H // 128    # 9
SPAN_DST = 127           # dst window per span; slot 127 = pad marker

# meta_idx layout (i16 cols): src_idx_a 0:72, src_idx_b 72:144, xr_idx 144:288
MI_A0, MI_A1 = 0, SLOT_H // 16
MI_B0, MI_B1 = MI_A1, 2 * (SLOT_H // 16)
MI_X0, MI_X1 = MI_B1, MI_B1 + SPAN_EDGES // 16
# meta_val layout (f16 cols): dcol 0:18, eac 18:36, gmask 36:40
MV_D0, MV_D1 = 0, NSG
MV_E0, MV_E1 = NSG, 2 * NSG
MV_G0, MV_G1 = 2 * NSG, 2 * NSG + 4
MV_W = MV_G1
# wpack rows (f16, width 256)
WP_WL1, WP_WR1 = 0, 65
WP_ATT1, WP_WE1, WP_BIAS1 = 130, 258, 386
WP_WL2, WP_WR2 = 514, 771
WP_ATT2, WP_WE2, WP_BIAS2 = 1028, 1156, 1284
WP_IOTA = 1412
WP_ROWS = 1540
# mlp rows (f32, width 128)
MP_P1, MP_LNG, MP_LNB, MP_P2, MP_CNT = 0, 257, 261, 265, 394
MP_ROWS = 398


# ----------------------------------------------------------------------------
# Host-side sharding / packing
# ----------------------------------------------------------------------------

def _host_prep(inp):
    x = np.asarray(inp["x"], dtype=np.float32)
    ei = np.asarray(inp["edge_index"], dtype=np.int32)
    ea_full = np.asarray(inp["edge_attr"], dtype=np.float32)[:, 0]
    batch = np.asarray(inp["batch"], dtype=np.int32)

    src0, dst0 = ei[0], ei[1]
    deg = np.maximum(np.bincount(dst0, minlength=N).astype(np.float64), 1.0)
    loop_attr = (
        np.bincount(dst0, weights=ea_full.astype(np.float64), minlength=N) / deg
    ).astype(np.float32)
    src = np.concatenate([src0, np.arange(N, dtype=np.int32)])
    dst = np.concatenate([dst0, np.arange(N, dtype=np.int32)])
    eattr = np.concatenate([ea_full, loop_attr]).astype(np.float32)

    gcounts = np.bincount(batch, minlength=NG)
    gstart = np.concatenate([[0], np.cumsum(gcounts)])
    core_n0 = np.array([gstart[4 * k] for k in range(NCORES)] + [N], dtype=np.int64)

    order = np.argsort(dst, kind="stable")
    src, dst, eattr = src[order], dst[order], eattr[order]
    edge_lo = np.searchsorted(dst, core_n0[:-1], "left")
    edge_hi = np.searchsorted(dst, core_n0[1:], "left")

    # src owner core (cores 0-3 -> table half A, 4-7 -> half B); stable
    # under span-count changes so it can drive packing.
    src_owner = np.searchsorted(core_n0[1:], src, "right")
    src_in_a = src_owner < (NCORES // 2)

    cores = []
    for k in range(NCORES):
        n0, n1 = int(core_n0[k]), int(core_n0[k + 1])
        s, e = int(edge_lo[k]), int(edge_hi[k])
        cd = dst[s:e]
        ca = src_in_a[s:e]
        nlocal = n1 - n0
        node_edge_start = np.searchsorted(cd, n0 + np.arange(nlocal + 1))
        cumA = np.concatenate([[0], np.cumsum(ca)])  # over edges
        spans = []
        b = 0
        while b < nlocal:
            bend = b
            while bend < nlocal and (bend - b) < SPAN_DST:
                e0, e1 = node_edge_start[b], node_edge_start[bend + 1]
                nA = cumA[e1] - cumA[e0]
                nB = (e1 - e0) - nA
                if nA > SLOT_H or nB > SLOT_H:
                    break
                bend += 1
            assert bend > b, "single node exceeds span edge capacity"
            spans.append(
                (b, bend - b, int(node_edge_start[b]), int(node_edge_start[bend]))
            )
            b = bend
        cores.append(
            dict(n0=n0, n1=n1, spans=spans, src=src[s:e], dst=cd, ea=eattr[s:e],
                 in_a=ca)
        )

    nspans = max(len(c["spans"]) for c in cores)
    rows_per_core = nspans * 128
    rows_total = NCORES * rows_per_core

    # global padded row per node
    node_row = np.zeros(N, dtype=np.int64)
    for k, c in enumerate(cores):
        for si, (b, nb, _, _) in enumerate(c["spans"]):
            nodes = np.arange(c["n0"] + b, c["n0"] + b + nb)
            node_row[nodes] = k * rows_per_core + si * 128 + (nodes - c["n0"] - b)

    # x padded, transposed, with ones row (for encoder rhs)
    x_pad = np.zeros((rows_total, 4), dtype=np.float32)
    x_pad[node_row] = x
    x_aug_T = np.concatenate(
        [x_pad.T, np.ones((1, rows_total), dtype=np.float32)], axis=0
    )  # [5, R]

    packs = []
    half_rows = rows_total // 2
    assert half_rows <= 32767, f"table half {half_rows} exceeds int16 index range"

    def wrap_idx16(vals):
        # vals: [SLOT] int -> wrapped [128, SLOT//16] int16 (16-part wrap,
        # replicated over the 8 q7 core groups)
        slot = len(vals)
        base = np.zeros((16, slot // 16), dtype=np.int16)
        i = np.arange(slot)
        base[i % 16, i // 16] = vals.astype(np.int16)
        return np.tile(base, (8, 1))

    for k, c in enumerate(cores):
        meta_idx = np.zeros((nspans, 128, MI_X1), dtype=np.int16)
        meta_val = np.zeros((nspans, 128, MV_W), dtype=np.float16)
        meta_val[:, :, MV_D0:MV_D1] = np.float16(127.0)
        for si, (b, nb, e0, e1) in enumerate(c["spans"]):
            ina = c["in_a"][e0:e1]
            esrc = node_row[c["src"][e0:e1]]
            edrel = (c["dst"][e0:e1] - c["n0"] - b).astype(np.int64)
            eea = c["ea"][e0:e1]
            # slots: A edges first (in region [0, SLOT_H)), then B edges at
            # [SLOT_H, 2*SLOT_H); pads keep idx 0 / drel 127 / ea 0
            ia = np.where(ina)[0]
            ib = np.where(~ina)[0]
            slots = np.empty(len(ina), dtype=np.int64)
            slots[ia] = np.arange(len(ia))
            slots[ib] = SLOT_H + np.arange(len(ib))
            av = np.zeros(SLOT_H, dtype=np.int64)
            av[:len(ia)] = esrc[ia]
            bv = np.zeros(SLOT_H, dtype=np.int64)
            bv[:len(ib)] = esrc[ib] - half_rows
            meta_idx[si, :, MI_A0:MI_A1] = wrap_idx16(av)
            meta_idx[si, :, MI_B0:MI_B1] = wrap_idx16(bv)
            xv = np.full(SPAN_EDGES, si * 128 + 127, dtype=np.int64)
            xv[slots] = si * 128 + edrel
            meta_idx[si, :, MI_X0:MI_X1] = wrap_idx16(xv)
            p, sg = slots % 128, slots // 128
            meta_val[si, p, MV_D0 + sg] = edrel.astype(np.float16)
            meta_val[si, p, MV_E0 + sg] = eea.astype(np.float16)
            nodes = np.arange(c["n0"] + b, c["n0"] + b + nb)
            gl = batch[nodes] - 4 * k
            meta_val[si, np.arange(nb), MV_G0 + gl] = np.float16(1.0)
        inv_cnt = np.zeros((4,), dtype=np.float32)
        for gg in range(4):
            cnt = max(int(gcounts[4 * k + gg]), 1)
            inv_cnt[gg] = 1.0 / cnt
        packs.append(
            dict(
                meta_idx=meta_idx,
                meta_val=meta_val,
                inv_cnt=inv_cnt,
                own_cols=np.arange(
                    k * rows_per_core, (k + 1) * rows_per_core, dtype=np.int64
                ),
            )
        )
    return cores, packs, nspans, rows_per_core, rows_total, x_aug_T, node_row


# ----------------------------------------------------------------------------
# Device program
# ----------------------------------------------------------------------------

_PROGRAM_CACHE = {}


def _build_program(nspans, rows_total, reps=1):
    rows_per_core = nspans * 128
    nblocks = rows_total // 128

    nc = bacc.Bacc()
    tcx = tile.TileContext(nc)

    t_feat = nc.dram_tensor(
        "feat", [5, rows_total + rows_per_core + 64], F32, kind="ExternalInput"
    )
    t_wpack = nc.dram_tensor("wpack", [WP_ROWS, HC], F16, kind="ExternalInput")
    t_midx = nc.dram_tensor(
        "meta_idx", [nspans, 128, MI_X1], I16, kind="ExternalInput"
    )
    t_mval = nc.dram_tensor(
        "meta_val", [nspans, 128, MV_W], F16, kind="ExternalInput"
    )
    t_mlp = nc.dram_tensor("mlp", [MP_ROWS, 128], F32, kind="ExternalInput")
    t_out = nc.dram_tensor("out", [4, 64], F32, kind="ExternalOutput")

    # ---- internal DRAM ----
    t_xl1 = nc.dram_tensor("xl1_tbl", [rows_total, HC], F16)
    t_xr1 = nc.dram_tensor("xr1_own", [rows_per_core, HC], F16)
    t_h1 = nc.dram_tensor("h1_own", [rows_per_core, HC], F16)
    t_xr2 = nc.dram_tensor("xr2_own", [rows_per_core, HC], F16)
    t_xl2_in = nc.dram_tensor("xl2_own_cc", [rows_per_core, HC], F16)
    t_xl2 = nc.dram_tensor("xl2_tbl", [rows_total, HC], F16, addr_space="Shared")

    from contextlib import ExitStack
    with tcx as tc, ExitStack() as es:
        # ------------------------------------------------------------------
        # constants in SBUF (loaded once, reused by every rep)
        # ------------------------------------------------------------------
        cpool = es.enter_context(tc.tile_pool(name="consts", bufs=1))
        enc_aug = cpool.tile([5, 64], F32)
        nc.sync.dma_start(
            out=enc_aug[:],
            in_=t_feat[:, rows_total + rows_per_core:rows_total + rows_per_core + 64],
        )
        iota_rep = cpool.tile([128, 128], F16)
        nc.sync.dma_start(out=iota_rep[:], in_=t_wpack[WP_IOTA:WP_IOTA + 128, 0:128])
        reps_t = {}
        for L, (r_att, r_we, r_bias) in (
            (1, (WP_ATT1, WP_WE1, WP_BIAS1)),
            (2, (WP_ATT2, WP_WE2, WP_BIAS2)),
        ):
            for nm, r0 in (("att_row", r_att), ("we_row", r_we), ("bias_row", r_bias)):
                rep = cpool.tile([128, HC], F16, tag=f"rep{L}{nm}")
                nc.sync.dma_start(out=rep[:], in_=t_wpack[r0:r0 + 128, :])
                reps_t[(L, nm)] = rep
        ones_col = cpool.tile([1, 128], F16)
        nc.vector.memset(ones_col[:], 1.0)

        wpool = es.enter_context(tc.tile_pool(name="weights", bufs=1))
        wl1 = wpool.tile([65, HC], F16)
        wr1 = wpool.tile([65, HC], F16)
        nc.sync.dma_start(out=wl1[:], in_=t_wpack[WP_WL1:WP_WL1 + 65, :])
        nc.sync.dma_start(out=wr1[:], in_=t_wpack[WP_WR1:WP_WR1 + 65, :])
        w2_tiles = {}
        for nm, r0 in (("wl_aug", WP_WL2), ("wr_aug", WP_WR2)):
            a = wpool.tile([128, HC], F16, tag=f"{nm}a")
            b = wpool.tile([128, HC], F16, tag=f"{nm}b")
            cbias = wpool.tile([1, HC], F16, tag=f"{nm}c")
            nc.sync.dma_start(out=a[:], in_=t_wpack[r0:r0 + 128, :])
            nc.sync.dma_start(out=b[:], in_=t_wpack[r0 + 128:r0 + 256, :])
            nc.sync.dma_start(out=cbias[:], in_=t_wpack[r0 + 256:r0 + 257, :])
            w2_tiles[nm] = (a, b, cbias)
        # MLP constants
        mpool = es.enter_context(tc.tile_pool(name="mlpc", bufs=1))
        p1a = mpool.tile([128, 128], F32)
        p1b = mpool.tile([128, 128], F32)
        p1c = mpool.tile([1, 128], F32)
        nc.sync.dma_start(out=p1a[:], in_=t_mlp[MP_P1:MP_P1 + 128, :])
        nc.sync.dma_start(out=p1b[:], in_=t_mlp[MP_P1 + 128:MP_P1 + 256, :])
        nc.sync.dma_start(out=p1c[:], in_=t_mlp[MP_P1 + 256:MP_P1 + 257, :])
        p2a = mpool.tile([128, 64], F32)
        p2c = mpool.tile([1, 64], F32)
        nc.sync.dma_start(out=p2a[:], in_=t_mlp[MP_P2:MP_P2 + 128, 0:64])
        nc.sync.dma_start(out=p2c[:], in_=t_mlp[MP_P2 + 128:MP_P2 + 129, 0:64])
        lng = mpool.tile([4, 128], F32)
        nc.sync.dma_start(out=lng[:], in_=t_mlp[MP_LNG:MP_LNG + 4, :])
        lnb = mpool.tile([4, 128], F32)
        nc.sync.dma_start(out=lnb[:], in_=t_mlp[MP_LNB:MP_LNB + 4, :])
        icnt = mpool.tile([4, 1], F32)
        nc.sync.dma_start(out=icnt[:], in_=t_mlp[MP_CNT:MP_CNT + 4, 0:1])
        ident = mpool.tile([128, 128], F32)
        from concourse.masks import make_identity
        make_identity(nc, ident[:])
        onesg = mpool.tile([1, 4], F32)
        nc.vector.memset(onesg[:], 1.0)

        def encode_block(pool, ppool, xaugT_ap):
            """xaugT_ap: [5, 128] f32 dram slice -> h0T_aug [65, 128] f16 tile"""
            xT = pool.tile([5, 128], F32, tag="xT")
            nc.sync.dma_start(out=xT[:], in_=xaugT_ap)
            h0psum = ppool.tile([64, 128], F32, tag="h0ps")
            nc.tensor.matmul(out=h0psum[:], lhsT=enc_aug[:], rhs=xT[:],
                             start=True, stop=True)
            h0T = pool.tile([65, 128], F16, tag="h0T")
            nc.scalar.activation(out=h0T[0:64, :], in_=h0psum[:], func=AF.Relu)
            nc.vector.tensor_copy(out=h0T[64:65, :], in_=ones_col[:])
            return h0T

        def build_rep():
            # --------------------------------------------------------------
            # Phase 1: encoder + xl1 for ALL rows
            # --------------------------------------------------------------
            with tc.tile_pool(name="p1", bufs=3) as pool, \
                 tc.tile_pool(name="p1ps", bufs=2, space="PSUM") as ppool:
                for blk in range(nblocks):
                    h0T = encode_block(pool, ppool,
                                       t_feat[:, blk * 128:(blk + 1) * 128])
                    xlp = ppool.tile([128, HC], F32, tag="xlps")
                    nc.tensor.matmul(out=xlp[:], lhsT=h0T[:], rhs=wl1[:],
                                     start=True, stop=True)
                    xls = pool.tile([128, HC], F16, tag="xls")
                    if blk % 2 == 0:
                        nc.vector.tensor_copy(out=xls[:], in_=xlp[:])
                    else:
                        nc.scalar.copy(out=xls[:], in_=xlp[:])
                    nc.sync.dma_start(
                        out=t_xl1[blk * 128:(blk + 1) * 128, :], in_=xls[:]
                    )

                # own xr1
                for s in range(nspans):
                    h0T = encode_block(
                        pool, ppool,
                        t_feat[:, rows_total + s * 128:rows_total + (s + 1) * 128],
                    )
                    xrp = ppool.tile([128, HC], F32, tag="xlps")
                    nc.tensor.matmul(out=xrp[:], lhsT=h0T[:], rhs=wr1[:],
                                     start=True, stop=True)
                    xrs = pool.tile([128, HC], F16, tag="xls")
                    nc.vector.tensor_copy(out=xrs[:], in_=xrp[:])
                    nc.sync.dma_start(
                        out=t_xr1[s * 128:(s + 1) * 128, :], in_=xrs[:]
                    )

            # --------------------------------------------------------------
            # GAT span loop (shared for both layers)
            # --------------------------------------------------------------
            def gat_layer(L, xl_tbl, xr_tbl, h_sink):
                """h_sink(s, htile, mval): consume flush output [128, HC] f16."""
                att_rep = reps_t[(L, "att_row")]
                we_rep = reps_t[(L, "we_row")]
                bias_rep = reps_t[(L, "bias_row")]
                with tc.tile_pool(name=f"g{L}", bufs=2) as pool, \
                     tc.tile_pool(name=f"g{L}c", bufs=1) as lpool, \
                     tc.tile_pool(name=f"g{L}b", bufs=3) as spool, \
                     tc.tile_pool(name=f"g{L}ps", bufs=2, space="PSUM") as ppool:
                    half_rows = rows_total // 2
                    # att row materialized across subgroups once per layer so
                    # the per-span z multiply runs as a plain contiguous TT
                    attB = lpool.tile([128, NSG, HC], F16)
                    nc.vector.tensor_copy(
                        out=attB[:],
                        in_=att_rep[:].rearrange(
                            "p (o c) -> p o c", o=1
                        ).broadcast_to((128, NSG, HC)),
                    )
                    for s in range(nspans):
                        midx = spool.tile([128, MI_X1], I16, tag="midx")
                        nc.sync.dma_start(out=midx[:], in_=t_midx[s, :, :])
                        mval = spool.tile([128, MV_W], F16, tag="mval")
                        nc.sync.dma_start(out=mval[:], in_=t_mval[s, :, :])
                        dcol = spool.tile([128, 2 * NSG], F32, tag="dcolF")
                        nc.vector.tensor_copy(out=dcol[:],
                                              in_=mval[:, MV_D0:MV_E1])
                        eac = dcol[:, NSG:2 * NSG]

                        # G = xl[src] (two half-table gathers), R = xr[dst],
                        # v = (we*ea + R) + G
                        G = pool.tile([128, NSG, HC], F16, tag="G")
                        nc.gpsimd.dma_gather(
                            G[:, 0:NSG_H, :], xl_tbl[0:half_rows, :],
                            midx[:, MI_A0:MI_A1],
                            SLOT_H, SLOT_H, HC, single_packet=False,
                        )
                        nc.gpsimd.dma_gather(
                            G[:, NSG_H:NSG, :], xl_tbl[half_rows:, :],
                            midx[:, MI_B0:MI_B1],
                            SLOT_H, SLOT_H, HC, single_packet=False,
                        )
                        R = pool.tile([128, NSG, HC], F16, tag="R")
                        nc.gpsimd.dma_gather(
                            R[:, :, :], xr_tbl[:, :], midx[:, MI_X0:MI_X1],
                            SPAN_EDGES, SPAN_EDGES, HC, single_packet=False,
                        )
                        v = pool.tile([128, NSG, HC], F16, tag="v")
                        for sg in range(NSG):
                            nc.vector.tensor_scalar(
                                out=v[:, sg, :], in0=we_rep[:],
                                scalar1=eac[:, sg:sg + 1], scalar2=None,
                                op0=ALU.mult,
                            )
                        nc.vector.tensor_tensor(
                            out=v[:, :, :], in0=v[:, :, :], in1=R[:, :, :],
                            op=ALU.add
                        )
                        nc.vector.tensor_tensor(
                            out=v[:, :, :], in0=v[:, :, :], in1=G[:, :, :],
                            op=ALU.add
                        )

                        # u = lrelu(v), z = u*att, alpha = per-head sum
                        u = pool.tile([128, NSG, HC], F16, tag="u")
                        nc.scalar.activation(out=u[:, :, :], in_=v[:, :, :],
                                             func=AF.Lrelu, alpha=0.2)
                        z = pool.tile([128, NSG, HC], F16, tag="z")
                        nc.vector.tensor_tensor(
                            out=z[:, :, :], in0=u[:, :, :], in1=attB[:],
                            op=ALU.mult
                        )
                        # per-head sums via binary fold tree
                        zf = pool.tile([128, NSG, 4, 32], F16, tag="zf")
                        z4 = z[:].rearrange("p s (h c) -> p s h c", h=4)
                        nc.vector.tensor_tensor(
                            out=zf[:, :, :, :], in0=z4[:, :, :, 0:32],
                            in1=z4[:, :, :, 32:64], op=ALU.add,
                        )
                        w = 16
                        while w >= 2:
                            nc.vector.tensor_tensor(
                                out=zf[:, :, :, 0:w], in0=zf[:, :, :, 0:w],
                                in1=zf[:, :, :, w:2 * w], op=ALU.add,
                            )
                            w //= 2
                        alpha = spool.tile([128, 4 * NSG], F32, tag="alpha")
                        nc.vector.tensor_tensor(
                            out=alpha[:].rearrange("p (s h o) -> p s h o",
                                                   h=4, o=1),
                            in0=zf[:, :, :, 0:1], in1=zf[:, :, :, 1:2],
                            op=ALU.add,
                        )
                        exF = spool.tile([128, 4 * NSG], F32, tag="exF")
                        nc.scalar.activation(out=exF[:], in_=alpha[:], func=AF.Exp)
                        ex = spool.tile([128, 4 * NSG], F16, tag="ex")
                        nc.vector.tensor_copy(out=ex[:], in_=exF[:])

                        # m2 = ex * G (softmax-weighted source messages;
                        # out = sum a*xl[src] directly, no xr/we correction)
                        m2 = pool.tile([128, NSG, HC], F16, tag="m2")
                        nc.vector.tensor_tensor(
                            out=m2[:],
                            in0=G[:].rearrange("p s (h c) -> p s h c", h=4),
                            in1=exF[:].rearrange(
                                "p (s h o) -> p s h o", h=4, o=1
                            ).broadcast_to((128, NSG, 4, C)),
                            op=ALU.mult,
                        )
                        S = pool.tile([128, NSG, 128], F16, tag="S")
                        accM = ppool.tile([128, HC], F32, tag="accM")
                        accE = ppool.tile([128, 4], F32, tag="accE")
                        for sg in range(NSG):
                            nc.vector.tensor_scalar(
                                out=S[:, sg, :], in0=iota_rep[:],
                                scalar1=dcol[:, sg:sg + 1], scalar2=None,
                                op0=ALU.is_equal,
                            )
                            nc.tensor.matmul(out=accM[:], lhsT=S[:, sg, :],
                                             rhs=m2[:, sg, :], start=(sg == 0),
                                             stop=(sg == NSG - 1))
                            nc.tensor.matmul(out=accE[:], lhsT=S[:, sg, :],
                                             rhs=ex[:, sg * 4:sg * 4 + 4],
                                             start=(sg == 0), stop=(sg == NSG - 1))

                        # flush: h = relu(accM/den + bias)
                        den = spool.tile([128, 4], F32, tag="den")
                        nc.vector.tensor_scalar(
                            out=den[:], in0=accE[:], scalar1=1e-30,
                            scalar2=None, op0=ALU.add,
                        )
                        rden = spool.tile([128, 4], F32, tag="rden")
                        nc.vector.reciprocal(out=rden[:], in_=den[:])
                        hT = spool.tile([128, HC], F16, tag="hT")
                        for hh in range(4):
                            blks = slice(hh * C, (hh + 1) * C)
                            nc.vector.scalar_tensor_tensor(
                                out=hT[:, blks], in0=accM[:, blks],
                                scalar=rden[:, hh:hh + 1], in1=bias_rep[:, blks],
                                op0=ALU.mult, op1=ALU.add,
                            )
                        hOut = spool.tile([128, HC], F16, tag="hOut")
                        nc.scalar.activation(out=hOut[:], in_=hT[:], func=AF.Relu)
                        h_sink(s, hOut, mval, pool, ppool)

            # layer 1: sink writes h1 to DRAM
            def h1_sink(s, hOut, mval, pool, ppool):
                nc.sync.dma_start(out=t_h1[s * 128:(s + 1) * 128, :], in_=hOut[:])

            gat_layer(1, t_xl1, t_xr1, h1_sink)

            # --------------------------------------------------------------
            # Phase 4: xl2/xr2 from h1 (own spans)
            # --------------------------------------------------------------
            with tc.tile_pool(name="p4", bufs=3) as pool, \
                 tc.tile_pool(name="p4ps", bufs=2, space="PSUM") as ppool:
                for s in range(nspans):
                    h1T0 = pool.tile([128, 128], F16, tag="h1T0")
                    h1T1 = pool.tile([128, 128], F16, tag="h1T1")
                    nc.sync.dma_start(
                        out=h1T0[:], in_=t_h1[s * 128:(s + 1) * 128, 0:128],
                        transpose=True,
                    )
                    nc.sync.dma_start(
                        out=h1T1[:], in_=t_h1[s * 128:(s + 1) * 128, 128:256],
                        transpose=True,
                    )
                    for nm, sink in (("wl_aug", t_xl2_in), ("wr_aug", t_xr2)):
                        wa, wb, wc = w2_tiles[nm]
                        ps = ppool.tile([128, HC], F32, tag="ps")
                        nc.tensor.matmul(out=ps[:], lhsT=h1T0[:], rhs=wa[:],
                                         start=True, stop=False)
                        nc.tensor.matmul(out=ps[:], lhsT=h1T1[:], rhs=wb[:],
                                         start=False, stop=False)
                        nc.tensor.matmul(out=ps[:], lhsT=ones_col[:],
                                         rhs=wc[:], start=False, stop=True)
                        xs = pool.tile([128, HC], F16, tag="xs")
                        nc.vector.tensor_copy(out=xs[:], in_=ps[:])
                        nc.sync.dma_start(out=sink[s * 128:(s + 1) * 128, :],
                                          in_=xs[:])

            # --------------------------------------------------------------
            # Phase 5: AllGather xl2
            # --------------------------------------------------------------
            nc.gpsimd.collective_compute(
                "AllGather",
                ALU.bypass,
                replica_groups=[list(range(NCORES))],
                ins=[t_xl2_in.ap().opt()],
                outs=[t_xl2.ap().opt()],
            )

            # --------------------------------------------------------------
            # Phase 6: GAT layer 2 with fused pooling
            # --------------------------------------------------------------
            with tc.tile_pool(name="gpool_ps", bufs=1, space="PSUM") as gpool_ps:
                gpsum = gpool_ps.tile([4, HC], F32)

                def h2_sink(s, hOut, mval, pool, ppool):
                    nc.tensor.matmul(out=gpsum[:], lhsT=mval[:, MV_G0:MV_G1],
                                     rhs=hOut[:],
                                     start=(s == 0), stop=(s == nspans - 1))

                gat_layer(2, t_xl2, t_xr2, h2_sink)

                # ----------------------------------------------------------
                # Phase 7: pooling -> MLP -> out
                # ----------------------------------------------------------
                with tc.tile_pool(name="mlp", bufs=1) as pool, \
                     tc.tile_pool(name="mlp_ps", bufs=2, space="PSUM") as ppool:
                    g = pool.tile([4, HC], F32)
                    nc.vector.tensor_scalar(out=g[:], in0=gpsum[:],
                                            scalar1=icnt[:, 0:1],
                                            scalar2=None, op0=ALU.mult)
                    # gT via PE transpose (two halves)
                    gT = pool.tile([128, 8], F32)
                    for half in range(2):
                        tp = ppool.tile([128, 128], F32, tag="tp")
                        nc.tensor.transpose(
                            out=tp[:, 0:4],
                            in_=g[:, half * 128:(half + 1) * 128],
                            identity=ident[0:4, 0:4],
                        )
                        nc.vector.tensor_copy(out=gT[:, half * 4:half * 4 + 4],
                                              in_=tp[:, 0:4])
                    z1p = ppool.tile([4, 128], F32, tag="z1p")
                    nc.tensor.matmul(out=z1p[:], lhsT=gT[:, 0:4], rhs=p1a[:],
                                     start=True, stop=False)
                    nc.tensor.matmul(out=z1p[:], lhsT=gT[:, 4:8], rhs=p1b[:],
                                     start=False, stop=False)
                    nc.tensor.matmul(out=z1p[:], lhsT=onesg[:], rhs=p1c[:],
                                     start=False, stop=True)
                    z1 = pool.tile([4, 128], F32)
                    nc.vector.tensor_copy(out=z1[:], in_=z1p[:])
                    # layernorm over free dim (128)
                    mu = pool.tile([4, 1], F32)
                    nc.vector.reduce_sum(out=mu[:], in_=z1[:], axis=AXX)
                    nc.vector.tensor_scalar(out=mu[:], in0=mu[:],
                                            scalar1=1.0 / 128,
                                            scalar2=None, op0=ALU.mult)
                    zc = pool.tile([4, 128], F32)
                    nc.vector.tensor_scalar(out=zc[:], in0=z1[:],
                                            scalar1=mu[:, 0:1],
                                            scalar2=None, op0=ALU.subtract)
                    sq = pool.tile([4, 128], F32)
                    nc.vector.tensor_tensor(out=sq[:], in0=zc[:], in1=zc[:],
                                            op=ALU.mult)
                    var = pool.tile([4, 1], F32)
                    nc.vector.reduce_sum(out=var[:], in_=sq[:], axis=AXX)
                    nc.vector.tensor_scalar(out=var[:], in0=var[:],
                                            scalar1=1.0 / 128,
                                            scalar2=1e-5, op0=ALU.mult,
                                            op1=ALU.add)
                    std = pool.tile([4, 1], F32)
                    nc.scalar.activation(out=std[:], in_=var[:], func=AF.Sqrt)
                    rstd = pool.tile([4, 1], F32)
                    nc.vector.reciprocal(out=rstd[:], in_=std[:])
                    zn = pool.tile([4, 128], F32)
                    nc.vector.tensor_scalar(out=zn[:], in0=zc[:],
                                            scalar1=rstd[:, 0:1],
                                            scalar2=None, op0=ALU.mult)
                    nc.vector.tensor_tensor(out=zn[:], in0=zn[:], in1=lng[:],
                                            op=ALU.mult)
                    nc.vector.tensor_tensor(out=zn[:], in0=zn[:], in1=lnb[:],
                                            op=ALU.add)
                    nc.scalar.activation(out=zn[:], in_=zn[:], func=AF.Relu)
                    # z2 = relu(zn @ p2 + b2)
                    znT = pool.tile([128, 4], F32)
                    tp2 = ppool.tile([128, 128], F32, tag="tp")
                    nc.tensor.transpose(out=tp2[:, 0:4], in_=zn[:],
                                        identity=ident[0:4, 0:4])
                    nc.vector.tensor_copy(out=znT[:], in_=tp2[:, 0:4])
                    z2p = ppool.tile([4, 64], F32, tag="z2p")
                    nc.tensor.matmul(out=z2p[:], lhsT=znT[:], rhs=p2a[:],
                                     start=True, stop=False)
                    nc.tensor.matmul(out=z2p[:], lhsT=onesg[:], rhs=p2c[:],
                                     start=False, stop=True)
                    zout = pool.tile([4, 64], F32)
                    nc.scalar.activation(out=zout[:], in_=z2p[:], func=AF.Relu)
                    nc.sync.dma_start(out=t_out[:], in_=zout[:])

        for _rep in range(reps):
            build_rep()

    nc.finalize()
    return nc


# ----------------------------------------------------------------------------
# Entry point
# ----------------------------------------------------------------------------

def _pack_inputs(inp, cores, packs, nspans, rows_per_core, rows_total, x_aug_T):
    f16 = np.float16
    f32 = np.float32
    # shared (replicated) blocks
    wpack = np.zeros((WP_ROWS, HC), dtype=f16)

    def aug(w, b):
        return np.concatenate(
            [np.asarray(w, f32), np.asarray(b, f32)[None, :]], 0
        ).astype(f16)

    wpack[WP_WL1:WP_WL1 + 65] = aug(inp["g1_wl"], inp["g1_bl"])
    wpack[WP_WR1:WP_WR1 + 65] = aug(inp["g1_wr"], inp["g1_br"])
    wpack[WP_WL2:WP_WL2 + 257] = aug(inp["g2_wl"], inp["g2_bl"])
    wpack[WP_WR2:WP_WR2 + 257] = aug(inp["g2_wr"], inp["g2_br"])
    for L, (r_att, r_we, r_bias) in (
        (1, (WP_ATT1, WP_WE1, WP_BIAS1)),
        (2, (WP_ATT2, WP_WE2, WP_BIAS2)),
    ):
        wpack[r_att:r_att + 128] = np.broadcast_to(
            np.asarray(inp[f"g{L}_att"], f32).reshape(1, HC), (128, HC)
        ).astype(f16)
        wpack[r_we:r_we + 128] = np.broadcast_to(
            np.asarray(inp[f"g{L}_we"], f32).reshape(1, HC), (128, HC)
        ).astype(f16)
        wpack[r_bias:r_bias + 128] = np.broadcast_to(
            np.asarray(inp[f"g{L}_bias"], f32).reshape(1, HC), (128, HC)
        ).astype(f16)
    wpack[WP_IOTA:WP_IOTA + 128, 0:128] = np.broadcast_to(
        np.arange(128, dtype=f16)[None, :], (128, 128)
    )

    mlp = np.zeros((MP_ROWS, 128), dtype=f32)
    mlp[MP_P1:MP_P1 + 257] = np.concatenate(
        [np.asarray(inp["p1_w"], f32), np.asarray(inp["p1_b"], f32)[None, :]], 0
    )
    mlp[MP_LNG:MP_LNG + 4] = np.asarray(inp["ln_g"], f32)[None, :]
    mlp[MP_LNB:MP_LNB + 4] = np.asarray(inp["ln_b"], f32)[None, :]
    mlp[MP_P2:MP_P2 + 129, 0:64] = np.concatenate(
        [np.asarray(inp["p2_w"], f32), np.asarray(inp["p2_b"], f32)[None, :]], 0
    )

    enc_aug = np.concatenate(
        [np.asarray(inp["enc_w"], f32), np.asarray(inp["enc_b"], f32)[None, :]], 0
    )  # [5, 64]

    in_maps = []
    for k in range(NCORES):
        p = packs[k]
        feat = np.zeros((5, rows_total + rows_per_core + 64), dtype=f32)
        feat[:, 0:rows_total] = x_aug_T
        feat[:, rows_total:rows_total + rows_per_core] = x_aug_T[:, p["own_cols"]]
        feat[:, rows_total + rows_per_core:] = enc_aug
        mlp_k = mlp.copy()
        mlp_k[MP_CNT:MP_CNT + 4, 0] = p["inv_cnt"]
        in_maps.append({
            "feat": feat,
            "wpack": wpack,
            "meta_idx": p["meta_idx"],
            "meta_val": p["meta_val"].view(np.float16),
            "mlp": mlp_k,
        })
    return in_maps


def kernel(**inputs):
    cores, packs, nspans, rows_per_core, rows_total, x_aug_T, node_row = _host_prep(
        inputs
    )
    key = (nspans, rows_total)
    if key not in _PROGRAM_CACHE:
        _PROGRAM_CACHE[key] = _build_program(nspans, rows_total)
    nc = _PROGRAM_CACHE[key]
    in_maps = _pack_inputs(
        inputs, cores, packs, nspans, rows_per_core, rows_total, x_aug_T
    )
    res = run_bass_kernel_spmd(nc, in_maps, core_ids=list(range(NCORES)))
    out = np.concatenate([res.results[k]["out"] for k in range(NCORES)], axis=0)
    return out.astype(np.float32)


if __name__ == "__main__":
    data = dict(np.load("/root/problem/inputs_cache.npz"))
    out = kernel(**data)
    exp = np.load("/root/problem/expected_np.npy")
    rel = np.linalg.norm(out - exp) / np.linalg.norm(exp)
    print("rel err:", rel)


# revision 20
# speedup vs baseline: 16.8843x; 1.2447x over previous
"""Trainium2 Bass kernel for nn_GATv2Base (gnn_message_passing).

Contract: kernel(**inputs) takes FULL unsharded inputs (same keys as
reference.setup_inputs()) and returns the FULL [32, 64] float32 output.

Sharding: 32 graphs -> 8 cores (4 graphs each, contiguous node ranges since
`batch` is sorted).  Edges (plus self-loops) are routed to the core owning
their dst node, sorted by dst, and packed into "spans" (<=127-node dst window,
2304 edge slots = 18 subgroups of 128 edges).  Node features live in a
span-major padded global layout so every per-span device address is static.
Layer 1 runs fully local (xl1 table computed replicated from x); between
layers one fp16 AllGather shares the xl2 table; the pooled per-graph MLP is
computed on the owning core.  Only the [4, 64] per-core outputs return to the
host.

All per-core inputs are packed into 5 device tensors (feat/wpack/meta_idx/
meta_val/mlp) to minimize per-call argument-marshalling overhead on the host
runtime.  _build_program(reps=N) unrolls the whole computation N times inside
one program; timing two variants isolates true device execution time from
dispatch overhead.
"""

import os
import sys

import numpy as np

for _p in ("/opt/trn_rl_repo", "/root/.axon_site/_ro/trn_rl_repo"):
    if os.path.isdir(_p) and _p not in sys.path:
        sys.path.insert(0, _p)

import concourse.bass as bass
import concourse.bacc as bacc
import concourse.mybir as mybir
import concourse.tile as tile
from concourse.bass_utils import run_bass_kernel_spmd

F32 = mybir.dt.float32
F16 = mybir.dt.float16
I16 = mybir.dt.int16
I32 = mybir.dt.int32
AF = mybir.ActivationFunctionType
ALU = mybir.AluOpType
AXX = mybir.AxisListType.X

N, E, H, C, NG = 50000, 800000, 4, 64, 32
HC = H * C
NCORES = 8
SLOT_H = 1152            # edge slots per table-half region (9 subgroups)
SPAN_EDGES = 2 * SLOT_H  # 2304 edge slots per span (18 subgroups of 128)
NSG = SPAN_EDGES // 128  # 18
NSG_H = SLOT_H // 128    # 9
SPAN_DST = 127           # dst window per span; slot 127 = pad marker

# meta_idx layout (i16 cols): src_idx_a 0:72, src_idx_b 72:144, xr_idx 144:288
MI_A0, MI_A1 = 0, SLOT_H // 16
MI_B0, MI_B1 = MI_A1, 2 * (SLOT_H // 16)
MI_X0, MI_X1 = MI_B1, MI_B1 + SPAN_EDGES // 16
# meta_val layout (f16 cols): dcol 0:18, eac 18:36, gmask 36:40
MV_D0, MV_D1 = 0, NSG
MV_E0, MV_E1 = NSG, 2 * NSG
MV_G0, MV_G1 = 2 * NSG, 2 * NSG + 4
MV_W = MV_G1
# wpack rows (f16, width 256)
WP_WL1, WP_WR1 = 0, 65
WP_ATT1, WP_WE1, WP_BIAS1 = 130, 258, 386
WP_WL2, WP_WR2 = 514, 771
WP_ATT2, WP_WE2, WP_BIAS2 = 1028, 1156, 1284
WP_IOTA = 1412
WP_ROWS = 1540
# mlp rows (f32, width 128)
MP_P1, MP_LNG, MP_LNB, MP_P2, MP_CNT = 0, 257, 261, 265, 394
MP_ROWS = 398


# ----------------------------------------------------------------------------
# Host-side sharding / packing
# ----------------------------------------------------------------------------

def _host_prep(inp):
    x = np.asarray(inp["x"], dtype=np.float32)
    ei = np.asarray(inp["edge_index"], dtype=np.int32)
    ea_full = np.asarray(inp["edge_attr"], dtype=np.float32)[:, 0]
    batch = np.asarray(inp["batch"], dtype=np.int32)

    src0, dst0 = ei[0], ei[1]
    deg = np.maximum(np.bincount(dst0, minlength=N).astype(np.float64), 1.0)
    loop_attr = (
        np.bincount(dst0, weights=ea_full.astype(np.float64), minlength=N) / deg
    ).astype(np.float32)
    src = np.concatenate([src0, np.arange(N, dtype=np.int32)])
    dst = np.concatenate([dst0, np.arange(N, dtype=np.int32)])
    eattr = np.concatenate([ea_full, loop_attr]).astype(np.float32)

    gcounts = np.bincount(batch, minlength=NG)
    gstart = np.concatenate([[0], np.cumsum(gcounts)])
    core_n0 = np.array([gstart[4 * k] for k in range(NCORES)] + [N], dtype=np.int64)

    order = np.argsort(dst, kind="stable")
    src, dst, eattr = src[order], dst[order], eattr[order]
    edge_lo = np.searchsorted(dst, core_n0[:-1], "left")
    edge_hi = np.searchsorted(dst, core_n0[1:], "left")

    # src owner core (cores 0-3 -> table half A, 4-7 -> half B); stable
    # under span-count changes so it can drive packing.
    src_owner = np.searchsorted(core_n0[1:], src, "right")
    src_in_a = src_owner < (NCORES // 2)

    cores = []
    for k in range(NCORES):
        n0, n1 = int(core_n0[k]), int(core_n0[k + 1])
        s, e = int(edge_lo[k]), int(edge_hi[k])
        cd = dst[s:e]
        ca = src_in_a[s:e]
        nlocal = n1 - n0
        node_edge_start = np.searchsorted(cd, n0 + np.arange(nlocal + 1))
        cumA = np.concatenate([[0], np.cumsum(ca)])  # over edges
        spans = []
        b = 0
        while b < nlocal:
            bend = b
            while bend < nlocal and (bend - b) < SPAN_DST:
                e0, e1 = node_edge_start[b], node_edge_start[bend + 1]
                nA = cumA[e1] - cumA[e0]
                nB = (e1 - e0) - nA
                if nA > SLOT_H or nB > SLOT_H:
                    break
                bend += 1
            assert bend > b, "single node exceeds span edge capacity"
            spans.append(
                (b, bend - b, int(node_edge_start[b]), int(node_edge_start[bend]))
            )
            b = bend
        cores.append(
            dict(n0=n0, n1=n1, spans=spans, src=src[s:e], dst=cd, ea=eattr[s:e],
                 in_a=ca)
        )

    nspans = max(len(c["spans"]) for c in cores)
    rows_per_core = nspans * 128
    rows_total = NCORES * rows_per_core

    # global padded row per node
    node_row = np.zeros(N, dtype=np.int64)
    for k, c in enumerate(cores):
        for si, (b, nb, _, _) in enumerate(c["spans"]):
            nodes = np.arange(c["n0"] + b, c["n0"] + b + nb)
            node_row[nodes] = k * rows_per_core + si * 128 + (nodes - c["n0"] - b)

    # x padded, transposed, with ones row (for encoder rhs)
    x_pad = np.zeros((rows_total, 4), dtype=np.float32)
    x_pad[node_row] = x
    x_aug_T = np.concatenate(
        [x_pad.T, np.ones((1, rows_total), dtype=np.float32)], axis=0
    )  # [5, R]

    packs = []
    half_rows = rows_total // 2
    assert half_rows <= 32767, f"table half {half_rows} exceeds int16 index range"

    def wrap_idx16(vals):
        # vals: [SLOT] int -> wrapped [128, SLOT//16] int16 (16-part wrap,
        # replicated over the 8 q7 core groups)
        slot = len(vals)
        base = np.zeros((16, slot // 16), dtype=np.int16)
        i = np.arange(slot)
        base[i % 16, i // 16] = vals.astype(np.int16)
        return np.tile(base, (8, 1))

    for k, c in enumerate(cores):
        meta_idx = np.zeros((nspans, 128, MI_X1), dtype=np.int16)
        meta_val = np.zeros((nspans, 128, MV_W), dtype=np.float16)
        meta_val[:, :, MV_D0:MV_D1] = np.float16(127.0)
        for si, (b, nb, e0, e1) in enumerate(c["spans"]):
            ina = c["in_a"][e0:e1]
            esrc = node_row[c["src"][e0:e1]]
            edrel = (c["dst"][e0:e1] - c["n0"] - b).astype(np.int64)
            eea = c["ea"][e0:e1]
            # slots: A edges first (in region [0, SLOT_H)), then B edges at
            # [SLOT_H, 2*SLOT_H); pads keep idx 0 / drel 127 / ea 0
            ia = np.where(ina)[0]
            ib = np.where(~ina)[0]
            slots = np.empty(len(ina), dtype=np.int64)
            slots[ia] = np.arange(len(ia))
            slots[ib] = SLOT_H + np.arange(len(ib))
            av = np.zeros(SLOT_H, dtype=np.int64)
            av[:len(ia)] = esrc[ia]
            bv = np.zeros(SLOT_H, dtype=np.int64)
            bv[:len(ib)] = esrc[ib] - half_rows
            meta_idx[si, :, MI_A0:MI_A1] = wrap_idx16(av)
            meta_idx[si, :, MI_B0:MI_B1] = wrap_idx16(bv)
            xv = np.full(SPAN_EDGES, si * 128 + 127, dtype=np.int64)
            xv[slots] = si * 128 + edrel
            meta_idx[si, :, MI_X0:MI_X1] = wrap_idx16(xv)
            p, sg = slots % 128, slots // 128
            meta_val[si, p, MV_D0 + sg] = edrel.astype(np.float16)
            meta_val[si, p, MV_E0 + sg] = eea.astype(np.float16)
            nodes = np.arange(c["n0"] + b, c["n0"] + b + nb)
            gl = batch[nodes] - 4 * k
            meta_val[si, np.arange(nb), MV_G0 + gl] = np.float16(1.0)
        inv_cnt = np.zeros((4,), dtype=np.float32)
        for gg in range(4):
            cnt = max(int(gcounts[4 * k + gg]), 1)
            inv_cnt[gg] = 1.0 / cnt
        packs.append(
            dict(
                meta_idx=meta_idx,
                meta_val=meta_val,
                inv_cnt=inv_cnt,
                own_cols=np.arange(
                    k * rows_per_core, (k + 1) * rows_per_core, dtype=np.int64
                ),
            )
        )
    return cores, packs, nspans, rows_per_core, rows_total, x_aug_T, node_row


# ----------------------------------------------------------------------------
# Device program
# ----------------------------------------------------------------------------

_PROGRAM_CACHE = {}


def _build_program(nspans, rows_total, reps=1, phase_limit=5, op_limit=9,
                   single_packet=False, nqueues=4, skip_r=False):
    rows_per_core = nspans * 128
    nblocks = rows_total // 128

    nc = bacc.Bacc(num_swdge_queues=nqueues)
    tcx = tile.TileContext(nc)

    t_feat = nc.dram_tensor(
        "feat", [5, rows_total + rows_per_core + 64], F32, kind="ExternalInput"
    )
    t_wpack = nc.dram_tensor("wpack", [WP_ROWS, HC], F16, kind="ExternalInput")
    t_midx = nc.dram_tensor(
        "meta_idx", [nspans, 128, MI_X1], I16, kind="ExternalInput"
    )
    t_mval = nc.dram_tensor(
        "meta_val", [nspans, 128, MV_W], F16, kind="ExternalInput"
    )
    t_mlp = nc.dram_tensor("mlp", [MP_ROWS, 128], F32, kind="ExternalInput")
    t_out = nc.dram_tensor("out", [4, 64], F32, kind="ExternalOutput")

    # ---- internal DRAM ----
    t_xl1 = nc.dram_tensor("xl1_tbl", [rows_total, HC], F16)
    t_xr1 = nc.dram_tensor("xr1_own", [rows_per_core, HC], F16)
    t_h1 = nc.dram_tensor("h1_own", [rows_per_core, HC], F16)
    t_xr2 = nc.dram_tensor("xr2_own", [rows_per_core, HC], F16)
    t_xl2_in = nc.dram_tensor("xl2_own_cc", [rows_per_core, HC], F16)
    t_xl2 = nc.dram_tensor("xl2_tbl", [rows_total, HC], F16, addr_space="Shared")

    from contextlib import ExitStack
    with tcx as tc, ExitStack() as es:
        # ------------------------------------------------------------------
        # constants in SBUF (loaded once, reused by every rep)
        # ------------------------------------------------------------------
        cpool = es.enter_context(tc.tile_pool(name="consts", bufs=1))
        enc_aug = cpool.tile([5, 64], F32)
        nc.sync.dma_start(
            out=enc_aug[:],
            in_=t_feat[:, rows_total + rows_per_core:rows_total + rows_per_core + 64],
        )
        iota_rep = cpool.tile([128, 128], F16)
        nc.sync.dma_start(out=iota_rep[:], in_=t_wpack[WP_IOTA:WP_IOTA + 128, 0:128])
        reps_t = {}
        for L, (r_att, r_we, r_bias) in (
            (1, (WP_ATT1, WP_WE1, WP_BIAS1)),
            (2, (WP_ATT2, WP_WE2, WP_BIAS2)),
        ):
            for nm, r0 in (("att_row", r_att), ("we_row", r_we), ("bias_row", r_bias)):
                rep = cpool.tile([128, HC], F16, tag=f"rep{L}{nm}")
                nc.sync.dma_start(out=rep[:], in_=t_wpack[r0:r0 + 128, :])
                reps_t[(L, nm)] = rep
        ones_col = cpool.tile([1, 128], F16)
        nc.vector.memset(ones_col[:], 1.0)
        ones_row = cpool.tile([1, 512], F16)
        nc.vector.memset(ones_row[:], 1.0)
        from concourse.masks import make_identity
        ident16 = cpool.tile([128, 128], F16)
        make_identity(nc, ident16[:])

        wpool = es.enter_context(tc.tile_pool(name="weights", bufs=1))
        wl1 = wpool.tile([65, HC], F16)
        wr1 = wpool.tile([65, HC], F16)
        nc.sync.dma_start(out=wl1[:], in_=t_wpack[WP_WL1:WP_WL1 + 65, :])
        nc.sync.dma_start(out=wr1[:], in_=t_wpack[WP_WR1:WP_WR1 + 65, :])
        w2_tiles = {}
        for nm, r0 in (("wl_aug", WP_WL2), ("wr_aug", WP_WR2)):
            a = wpool.tile([128, HC], F16, tag=f"{nm}a")
            b = wpool.tile([128, HC], F16, tag=f"{nm}b")
            cbias = wpool.tile([1, HC], F16, tag=f"{nm}c")
            nc.sync.dma_start(out=a[:], in_=t_wpack[r0:r0 + 128, :])
            nc.sync.dma_start(out=b[:], in_=t_wpack[r0 + 128:r0 + 256, :])
            nc.sync.dma_start(out=cbias[:], in_=t_wpack[r0 + 256:r0 + 257, :])
            w2_tiles[nm] = (a, b, cbias)
        # MLP constants
        mpool = es.enter_context(tc.tile_pool(name="mlpc", bufs=1))
        p1a = mpool.tile([128, 128], F32)
        p1b = mpool.tile([128, 128], F32)
        p1c = mpool.tile([1, 128], F32)
        nc.sync.dma_start(out=p1a[:], in_=t_mlp[MP_P1:MP_P1 + 128, :])
        nc.sync.dma_start(out=p1b[:], in_=t_mlp[MP_P1 + 128:MP_P1 + 256, :])
        nc.sync.dma_start(out=p1c[:], in_=t_mlp[MP_P1 + 256:MP_P1 + 257, :])
        p2a = mpool.tile([128, 64], F32)
        p2c = mpool.tile([1, 64], F32)
        nc.sync.dma_start(out=p2a[:], in_=t_mlp[MP_P2:MP_P2 + 128, 0:64])
        nc.sync.dma_start(out=p2c[:], in_=t_mlp[MP_P2 + 128:MP_P2 + 129, 0:64])
        lng = mpool.tile([4, 128], F32)
        nc.sync.dma_start(out=lng[:], in_=t_mlp[MP_LNG:MP_LNG + 4, :])
        lnb = mpool.tile([4, 128], F32)
        nc.sync.dma_start(out=lnb[:], in_=t_mlp[MP_LNB:MP_LNB + 4, :])
        icnt = mpool.tile([4, 1], F32)
        nc.sync.dma_start(out=icnt[:], in_=t_mlp[MP_CNT:MP_CNT + 4, 0:1])
        ident = mpool.tile([128, 128], F32)
        from concourse.masks import make_identity
        make_identity(nc, ident[:])
        onesg = mpool.tile([1, 4], F32)
        nc.vector.memset(onesg[:], 1.0)

        def encode4(pool, ppool, col0, ncols):
            """Encode ncols (<=512) padded nodes starting at feat col col0.
            Returns h0T4 [65, ncols] f16 (aug ones row included)."""
            xT = pool.tile([5, 512], F32, tag="xT")
            nc.sync.dma_start(out=xT[:, 0:ncols], in_=t_feat[:, col0:col0 + ncols])
            h0p = ppool.tile([64, 512], F32, tag="h0ps")
            nc.tensor.matmul(out=h0p[:, 0:ncols], lhsT=enc_aug[:],
                             rhs=xT[:, 0:ncols], start=True, stop=True)
            h0T = pool.tile([65, 512], F16, tag="h0T")
            nc.scalar.activation(out=h0T[0:64, 0:ncols], in_=h0p[:, 0:ncols],
                                 func=AF.Relu)
            nc.vector.tensor_copy(out=h0T[64:65, 0:ncols],
                                  in_=ones_row[:, 0:ncols])
            return h0T

        def xw_blocks(pool, ppool, h0T, w, nblk, sink_ap):
            """nblk xl/xr matmuls from h0T slices; one batched DMA to DRAM."""
            xls = pool.tile([128, 4, HC], F16, tag="xls")
            for j in range(nblk):
                xlp = ppool.tile([128, HC], F32, tag="xlps")
                nc.tensor.matmul(out=xlp[:], lhsT=h0T[:, j * 128:(j + 1) * 128],
                                 rhs=w[:], start=True, stop=True)
                if j % 2 == 0:
                    nc.vector.tensor_copy(out=xls[:, j, :], in_=xlp[:])
                else:
                    nc.scalar.copy(out=xls[:, j, :], in_=xlp[:])
            nc.sync.dma_start(
                out=sink_ap.rearrange("(b p) c -> p b c", p=128),
                in_=xls[:, 0:nblk, :],
            )

        def build_rep():
            # --------------------------------------------------------------
            # Phase 1: encoder + xl1 for ALL rows (4 blocks per DMA batch)
            # --------------------------------------------------------------
            with tc.tile_pool(name="p1", bufs=3) as pool, \
                 tc.tile_pool(name="p1ps", bufs=2, space="PSUM") as ppool:
                nb4 = (nblocks + 3) // 4 if phase_limit >= 1 else 0
                for b4 in range(nb4):
                    nblk = min(4, nblocks - b4 * 4)
                    h0T = encode4(pool, ppool, b4 * 512, nblk * 128)
                    xw_blocks(pool, ppool, h0T, wl1, nblk,
                              t_xl1[b4 * 512:b4 * 512 + nblk * 128, :])
                ns4 = (nspans + 3) // 4 if phase_limit >= 1 else 0
                for s4 in range(ns4):
                    nblk = min(4, nspans - s4 * 4)
                    h0T = encode4(pool, ppool, rows_total + s4 * 512, nblk * 128)
                    xw_blocks(pool, ppool, h0T, wr1, nblk,
                              t_xr1[s4 * 512:s4 * 512 + nblk * 128, :])

            # --------------------------------------------------------------
            # GAT span loop (shared for both layers)
            # --------------------------------------------------------------
            def gat_layer(L, xl_tbl, xr_tbl, h_sink):
                """h_sink(s, htile, mval): consume flush output [128, HC] f16."""
                att_rep = reps_t[(L, "att_row")]
                we_rep = reps_t[(L, "we_row")]
                bias_rep = reps_t[(L, "bias_row")]
                with tc.tile_pool(name=f"g{L}", bufs=2) as pool, \
                     tc.tile_pool(name=f"g{L}c", bufs=1) as lpool, \
                     tc.tile_pool(name=f"g{L}b", bufs=3) as spool, \
                     tc.tile_pool(name=f"g{L}t", bufs=2, space="PSUM") as tpool, \
                     tc.tile_pool(name=f"g{L}r", bufs=2, space="PSUM") as rpool, \
                     tc.tile_pool(name=f"g{L}ps", bufs=2, space="PSUM") as ppool:
                    half_rows = rows_total // 2
                    # att row materialized across subgroups once per layer so
                    # the per-span z multiply runs as a plain contiguous TT
                    attB = lpool.tile([128, NSG, HC], F16)
                    nc.vector.tensor_copy(
                        out=attB[:],
                        in_=att_rep[:].rearrange(
                            "p (o c) -> p o c", o=1
                        ).broadcast_to((128, NSG, HC)),
                    )
                    for s in range(nspans):
                        midx = spool.tile([128, MI_B1], I16, tag="midx")
                        nc.sync.dma_start(out=midx[:],
                                          in_=t_midx[s, :, 0:MI_B1])
                        mval = spool.tile([128, MV_W], F16, tag="mval")
                        nc.sync.dma_start(out=mval[:], in_=t_mval[s, :, :])
                        dcol = spool.tile([128, 2 * NSG], F32, tag="dcolF")
                        nc.vector.tensor_copy(out=dcol[:],
                                              in_=mval[:, MV_D0:MV_E1])
                        eac = dcol[:, NSG:2 * NSG]
                        xr_fl = spool.tile([128, HC], F16, tag="xrfl")
                        nc.sync.dma_start(
                            out=xr_fl[:], in_=xr_tbl[s * 128:(s + 1) * 128, :]
                        )

                        # G = xl[src] (two half-table gathers), R = xr[dst],
                        # v = (we*ea + R) + G
                        G = pool.tile([128, NSG, HC], F16, tag="G")
                        nc.gpsimd.dma_gather(
                            G[:, 0:NSG_H, :], xl_tbl[0:half_rows, :],
                            midx[:, MI_A0:MI_A1],
                            SLOT_H, SLOT_H, HC, single_packet=single_packet,
                            queue_num=0,
                        )
                        nc.gpsimd.dma_gather(
                            G[:, NSG_H:NSG, :], xl_tbl[half_rows:, :],
                            midx[:, MI_B0:MI_B1],
                            SLOT_H, SLOT_H, HC, single_packet=single_packet,
                            queue_num=1 % nqueues,
                        )
                        # R = xr[dst] expanded on-chip: per subgroup,
                        # transpose the dst one-hot S on PE, then one matmul
                        # ST @ xr_fl broadcasts the window rows to edge slots
                        # (replaces a 2304-descriptor DMA gather)
                        S = pool.tile([128, NSG, 128], F16, tag="S")
                        R = pool.tile([128, NSG, HC], F16, tag="R")
                        for sg in range(NSG):
                            nc.vector.tensor_scalar(
                                out=S[:, sg, :], in0=iota_rep[:],
                                scalar1=dcol[:, sg:sg + 1], scalar2=None,
                                op0=ALU.is_equal,
                            )
                            stp = tpool.tile([128, 128], F16, tag="stp")
                            nc.tensor.transpose(out=stp[:], in_=S[:, sg, :],
                                                identity=ident16[:])
                            sts = spool.tile([128, 128], F16, tag="sts")
                            nc.vector.tensor_copy(out=sts[:], in_=stp[:])
                            rp = rpool.tile([128, HC], F32, tag="rp")
                            nc.tensor.matmul(out=rp[:], lhsT=sts[:],
                                             rhs=xr_fl[:], start=True, stop=True)
                            nc.scalar.copy(out=R[:, sg, :], in_=rp[:])
                        if op_limit < 2:
                            hOut = spool.tile([128, HC], F16, tag="hOut")
                            nc.vector.tensor_copy(out=hOut[:], in_=G[:, 0, :])
                            h_sink(s, hOut, mval, pool, ppool)
                            continue
                        v = pool.tile([128, NSG, HC], F16, tag="v")
                        for sg in range(NSG):
                            nc.vector.tensor_scalar(
                                out=v[:, sg, :], in0=we_rep[:],
                                scalar1=eac[:, sg:sg + 1], scalar2=None,
                                op0=ALU.mult,
                            )
                        nc.vector.tensor_tensor(
                            out=v[:, :, :], in0=v[:, :, :], in1=R[:, :, :],
                            op=ALU.add
                        )
                        nc.vector.tensor_tensor(
                            out=v[:, :, :], in0=v[:, :, :], in1=G[:, :, :],
                            op=ALU.add
                        )

                        if op_limit < 3:
                            hOut = spool.tile([128, HC], F16, tag="hOut")
                            nc.vector.tensor_copy(out=hOut[:], in_=v[:, 0, :])
                            h_sink(s, hOut, mval, pool, ppool)
                            continue
                        # u = lrelu(v), z = u*att, alpha = per-head sum
                        u = pool.tile([128, NSG, HC], F16, tag="u")
                        nc.scalar.activation(out=u[:, :, :], in_=v[:, :, :],
                                             func=AF.Lrelu, alpha=0.2)
                        z = pool.tile([128, NSG, HC], F16, tag="z")
                        nc.vector.tensor_tensor(
                            out=z[:, :, :], in0=u[:, :, :], in1=attB[:],
                            op=ALU.mult
                        )
                        # per-head sums via binary fold tree
                        zf = pool.tile([128, NSG, 4, 32], F16, tag="zf")
                        z4 = z[:].rearrange("p s (h c) -> p s h c", h=4)
                        nc.vector.tensor_tensor(
                            out=zf[:, :, :, :], in0=z4[:, :, :, 0:32],
                            in1=z4[:, :, :, 32:64], op=ALU.add,
                        )
                        w = 16
                        while w >= 2:
                            nc.vector.tensor_tensor(
                                out=zf[:, :, :, 0:w], in0=zf[:, :, :, 0:w],
                                in1=zf[:, :, :, w:2 * w], op=ALU.add,
                            )
                            w //= 2
                        alpha = spool.tile([128, 4 * NSG], F32, tag="alpha")
                        nc.vector.tensor_tensor(
                            out=alpha[:].rearrange("p (s h o) -> p s h o",
                                                   h=4, o=1),
                            in0=zf[:, :, :, 0:1], in1=zf[:, :, :, 1:2],
                            op=ALU.add,
                        )
                        exF = spool.tile([128, 4 * NSG], F32, tag="exF")
                        nc.scalar.activation(out=exF[:], in_=alpha[:], func=AF.Exp)
                        ex = spool.tile([128, 4 * NSG], F16, tag="ex")
                        nc.vector.tensor_copy(out=ex[:], in_=exF[:])

                        if op_limit < 4:
                            hOut = spool.tile([128, HC], F16, tag="hOut")
                            nc.vector.tensor_copy(out=hOut[:], in_=v[:, 0, :])
                            nc.vector.tensor_scalar(
                                out=hOut[:, 0:4], in0=ex[:, 0:4], scalar1=1.0,
                                scalar2=None, op0=ALU.mult)
                            h_sink(s, hOut, mval, pool, ppool)
                            continue
                        # m2 = ex * G (softmax-weighted source messages;
                        # out = sum a*xl[src] directly, no xr/we correction)
                        m2 = pool.tile([128, NSG, HC], F16, tag="m2")
                        nc.vector.tensor_tensor(
                            out=m2[:],
                            in0=G[:].rearrange("p s (h c) -> p s h c", h=4),
                            in1=exF[:].rearrange(
                                "p (s h o) -> p s h o", h=4, o=1
                            ).broadcast_to((128, NSG, 4, C)),
                            op=ALU.mult,
                        )
                        accM = ppool.tile([128, HC], F32, tag="accM")
                        accE = ppool.tile([128, 4], F32, tag="accE", bufs=1)
                        for sg in range(NSG):
                            nc.tensor.matmul(out=accM[:], lhsT=S[:, sg, :],
                                             rhs=m2[:, sg, :], start=(sg == 0),
                                             stop=(sg == NSG - 1))
                            nc.tensor.matmul(out=accE[:], lhsT=S[:, sg, :],
                                             rhs=ex[:, sg * 4:sg * 4 + 4],
                                             start=(sg == 0), stop=(sg == NSG - 1))

                        if op_limit < 5:
                            hOut = spool.tile([128, HC], F16, tag="hOut")
                            nc.vector.tensor_copy(out=hOut[:], in_=accM[:])
                            h_sink(s, hOut, mval, pool, ppool)
                            continue
                        # flush: h = relu(accM/den + bias)
                        den = spool.tile([128, 4], F32, tag="den")
                        nc.vector.tensor_scalar(
                            out=den[:], in0=accE[:], scalar1=1e-30,
                            scalar2=None, op0=ALU.add,
                        )
                        rden = spool.tile([128, 4], F32, tag="rden")
                        nc.vector.reciprocal(out=rden[:], in_=den[:])
                        hT = spool.tile([128, HC], F16, tag="hT")
                        for hh in range(4):
                            blks = slice(hh * C, (hh + 1) * C)
                            nc.vector.scalar_tensor_tensor(
                                out=hT[:, blks], in0=accM[:, blks],
                                scalar=rden[:, hh:hh + 1], in1=bias_rep[:, blks],
                                op0=ALU.mult, op1=ALU.add,
                            )
                        hOut = spool.tile([128, HC], F16, tag="hOut")
                        nc.scalar.activation(out=hOut[:], in_=hT[:], func=AF.Relu)
                        h_sink(s, hOut, mval, pool, ppool)

            # layer 1: sink writes h1 to DRAM
            def h1_sink(s, hOut, mval, pool, ppool):
                nc.sync.dma_start(out=t_h1[s * 128:(s + 1) * 128, :], in_=hOut[:])

            if phase_limit >= 2:
                gat_layer(1, t_xl1, t_xr1, h1_sink)

            # --------------------------------------------------------------
            # Phase 4: xl2/xr2 from h1 (own spans)
            # --------------------------------------------------------------
            with tc.tile_pool(name="p4", bufs=3) as pool, \
                 tc.tile_pool(name="p4ps", bufs=2, space="PSUM") as ppool:
                for s in range(nspans if phase_limit >= 3 else 0):
                    h1T0 = pool.tile([128, 128], F16, tag="h1T0")
                    h1T1 = pool.tile([128, 128], F16, tag="h1T1")
                    nc.sync.dma_start(
                        out=h1T0[:], in_=t_h1[s * 128:(s + 1) * 128, 0:128],
                        transpose=True,
                    )
                    nc.sync.dma_start(
                        out=h1T1[:], in_=t_h1[s * 128:(s + 1) * 128, 128:256],
                        transpose=True,
                    )
                    for nm, sink in (("wl_aug", t_xl2_in), ("wr_aug", t_xr2)):
                        wa, wb, wc = w2_tiles[nm]
                        ps = ppool.tile([128, HC], F32, tag="ps")
                        nc.tensor.matmul(out=ps[:], lhsT=h1T0[:], rhs=wa[:],
                                         start=True, stop=False)
                        nc.tensor.matmul(out=ps[:], lhsT=h1T1[:], rhs=wb[:],
                                         start=False, stop=False)
                        nc.tensor.matmul(out=ps[:], lhsT=ones_col[:],
                                         rhs=wc[:], start=False, stop=True)
                        xs = pool.tile([128, HC], F16, tag="xs")
                        nc.vector.tensor_copy(out=xs[:], in_=ps[:])
                        nc.sync.dma_start(out=sink[s * 128:(s + 1) * 128, :],
                                          in_=xs[:])

            # --------------------------------------------------------------
            # Phase 5: AllGather xl2
            # --------------------------------------------------------------
            if phase_limit >= 4:
              nc.gpsimd.collective_compute(
                "AllGather",
                ALU.bypass,
                replica_groups=[list(range(NCORES))],
                ins=[t_xl2_in.ap().opt()],
                outs=[t_xl2.ap().opt()],
              )

            # --------------------------------------------------------------
            # Phase 6: GAT layer 2 with fused pooling
            # --------------------------------------------------------------
            if phase_limit < 5:
                with tc.tile_pool(name="dummyout", bufs=1) as dpool:
                    dz = dpool.tile([4, 64], F32)
                    nc.vector.memset(dz[:], 0.0)
                    nc.sync.dma_start(out=t_out[:], in_=dz[:])
                return
            with tc.tile_pool(name="gpool_ps", bufs=1, space="PSUM") as gpool_ps:
                gpsum = gpool_ps.tile([4, HC], F32)

                def h2_sink(s, hOut, mval, pool, ppool):
                    nc.tensor.matmul(out=gpsum[:], lhsT=mval[:, MV_G0:MV_G1],
                                     rhs=hOut[:],
                                     start=(s == 0), stop=(s == nspans - 1))

                gat_layer(2, t_xl2, t_xr2, h2_sink)

                # ----------------------------------------------------------
                # Phase 7: pooling -> MLP -> out
                # ----------------------------------------------------------
                with tc.tile_pool(name="mlp", bufs=1) as pool, \
                     tc.tile_pool(name="mlp_ps", bufs=2, space="PSUM") as ppool:
                    g = pool.tile([4, HC], F32)
                    nc.vector.tensor_scalar(out=g[:], in0=gpsum[:],
                                            scalar1=icnt[:, 0:1],
                                            scalar2=None, op0=ALU.mult)
                    # gT via PE transpose (two halves)
                    gT = pool.tile([128, 8], F32)
                    for half in range(2):
                        tp = ppool.tile([128, 128], F32, tag="tp")
                        nc.tensor.transpose(
                            out=tp[:, 0:4],
                            in_=g[:, half * 128:(half + 1) * 128],
                            identity=ident[0:4, 0:4],
                        )
                        nc.vector.tensor_copy(out=gT[:, half * 4:half * 4 + 4],
                                              in_=tp[:, 0:4])
                    z1p = ppool.tile([4, 128], F32, tag="z1p")
                    nc.tensor.matmul(out=z1p[:], lhsT=gT[:, 0:4], rhs=p1a[:],
                                     start=True, stop=False)
                    nc.tensor.matmul(out=z1p[:], lhsT=gT[:, 4:8], rhs=p1b[:],
                                     start=False, stop=False)
                    nc.tensor.matmul(out=z1p[:], lhsT=onesg[:], rhs=p1c[:],
                                     start=False, stop=True)
                    z1 = pool.tile([4, 128], F32)
                    nc.vector.tensor_copy(out=z1[:], in_=z1p[:])
                    # layernorm over free dim (128)
                    mu = pool.tile([4, 1], F32)
                    nc.vector.reduce_sum(out=mu[:], in_=z1[:], axis=AXX)
                    nc.vector.tensor_scalar(out=mu[:], in0=mu[:],
                                            scalar1=1.0 / 128,
                                            scalar2=None, op0=ALU.mult)
                    zc = pool.tile([4, 128], F32)
                    nc.vector.tensor_scalar(out=zc[:], in0=z1[:],
                                            scalar1=mu[:, 0:1],
                                            scalar2=None, op0=ALU.subtract)
                    sq = pool.tile([4, 128], F32)
                    nc.vector.tensor_tensor(out=sq[:], in0=zc[:], in1=zc[:],
                                            op=ALU.mult)
                    var = pool.tile([4, 1], F32)
                    nc.vector.reduce_sum(out=var[:], in_=sq[:], axis=AXX)
                    nc.vector.tensor_scalar(out=var[:], in0=var[:],
                                            scalar1=1.0 / 128,
                                            scalar2=1e-5, op0=ALU.mult,
                                            op1=ALU.add)
                    std = pool.tile([4, 1], F32)
                    nc.scalar.activation(out=std[:], in_=var[:], func=AF.Sqrt)
                    rstd = pool.tile([4, 1], F32)
                    nc.vector.reciprocal(out=rstd[:], in_=std[:])
                    zn = pool.tile([4, 128], F32)
                    nc.vector.tensor_scalar(out=zn[:], in0=zc[:],
                                            scalar1=rstd[:, 0:1],
                                            scalar2=None, op0=ALU.mult)
                    nc.vector.tensor_tensor(out=zn[:], in0=zn[:], in1=lng[:],
                                            op=ALU.mult)
                    nc.vector.tensor_tensor(out=zn[:], in0=zn[:], in1=lnb[:],
                                            op=ALU.add)
                    nc.scalar.activation(out=zn[:], in_=zn[:], func=AF.Relu)
                    # z2 = relu(zn @ p2 + b2)
                    znT = pool.tile([128, 4], F32)
                    tp2 = ppool.tile([128, 128], F32, tag="tp")
                    nc.tensor.transpose(out=tp2[:, 0:4], in_=zn[:],
                                        identity=ident[0:4, 0:4])
                    nc.vector.tensor_copy(out=znT[:], in_=tp2[:, 0:4])
                    z2p = ppool.tile([4, 64], F32, tag="z2p")
                    nc.tensor.matmul(out=z2p[:], lhsT=znT[:], rhs=p2a[:],
                                     start=True, stop=False)
                    nc.tensor.matmul(out=z2p[:], lhsT=onesg[:], rhs=p2c[:],
                                     start=False, stop=True)
                    zout = pool.tile([4, 64], F32)
                    nc.scalar.activation(out=zout[:], in_=z2p[:], func=AF.Relu)
                    nc.sync.dma_start(out=t_out[:], in_=zout[:])

        for _rep in range(reps):
            build_rep()

    nc.finalize()
    return nc


# ----------------------------------------------------------------------------
# Entry point
# ----------------------------------------------------------------------------

def _pack_inputs(inp, cores, packs, nspans, rows_per_core, rows_total, x_aug_T):
    f16 = np.float16
    f32 = np.float32
    # shared (replicated) blocks
    wpack = np.zeros((WP_ROWS, HC), dtype=f16)

    def aug(w, b):
        return np.concatenate(
            [np.asarray(w, f32), np.asarray(b, f32)[None, :]], 0
        ).astype(f16)

    wpack[WP_WL1:WP_WL1 + 65] = aug(inp["g1_wl"], inp["g1_bl"])
    wpack[WP_WR1:WP_WR1 + 65] = aug(inp["g1_wr"], inp["g1_br"])
    wpack[WP_WL2:WP_WL2 + 257] = aug(inp["g2_wl"], inp["g2_bl"])
    wpack[WP_WR2:WP_WR2 + 257] = aug(inp["g2_wr"], inp["g2_br"])
    for L, (r_att, r_we, r_bias) in (
        (1, (WP_ATT1, WP_WE1, WP_BIAS1)),
        (2, (WP_ATT2, WP_WE2, WP_BIAS2)),
    ):
        wpack[r_att:r_att + 128] = np.broadcast_to(
            np.asarray(inp[f"g{L}_att"], f32).reshape(1, HC), (128, HC)
        ).astype(f16)
        wpack[r_we:r_we + 128] = np.broadcast_to(
            np.asarray(inp[f"g{L}_we"], f32).reshape(1, HC), (128, HC)
        ).astype(f16)
        wpack[r_bias:r_bias + 128] = np.broadcast_to(
            np.asarray(inp[f"g{L}_bias"], f32).reshape(1, HC), (128, HC)
        ).astype(f16)
    wpack[WP_IOTA:WP_IOTA + 128, 0:128] = np.broadcast_to(
        np.arange(128, dtype=f16)[None, :], (128, 128)
    )

    mlp = np.zeros((MP_ROWS, 128), dtype=f32)
    mlp[MP_P1:MP_P1 + 257] = np.concatenate(
        [np.asarray(inp["p1_w"], f32), np.asarray(inp["p1_b"], f32)[None, :]], 0
    )
    mlp[MP_LNG:MP_LNG + 4] = np.asarray(inp["ln_g"], f32)[None, :]
    mlp[MP_LNB:MP_LNB + 4] = np.asarray(inp["ln_b"], f32)[None, :]
    mlp[MP_P2:MP_P2 + 129, 0:64] = np.concatenate(
        [np.asarray(inp["p2_w"], f32), np.asarray(inp["p2_b"], f32)[None, :]], 0
    )

    enc_aug = np.concatenate(
        [np.asarray(inp["enc_w"], f32), np.asarray(inp["enc_b"], f32)[None, :]], 0
    )  # [5, 64]

    in_maps = []
    for k in range(NCORES):
        p = packs[k]
        feat = np.zeros((5, rows_total + rows_per_core + 64), dtype=f32)
        feat[:, 0:rows_total] = x_aug_T
        feat[:, rows_total:rows_total + rows_per_core] = x_aug_T[:, p["own_cols"]]
        feat[:, rows_total + rows_per_core:] = enc_aug
        mlp_k = mlp.copy()
        mlp_k[MP_CNT:MP_CNT + 4, 0] = p["inv_cnt"]
        in_maps.append({
            "feat": feat,
            "wpack": wpack,
            "meta_idx": p["meta_idx"],
            "meta_val": p["meta_val"].view(np.float16),
            "mlp": mlp_k,
        })
    return in_maps


def kernel(**inputs):
    cores, packs, nspans, rows_per_core, rows_total, x_aug_T, node_row = _host_prep(
        inputs
    )
    key = (nspans, rows_total)
    if key not in _PROGRAM_CACHE:
        _PROGRAM_CACHE[key] = _build_program(nspans, rows_total)
    nc = _PROGRAM_CACHE[key]
    in_maps = _pack_inputs(
        inputs, cores, packs, nspans, rows_per_core, rows_total, x_aug_T
    )
    res = run_bass_kernel_spmd(nc, in_maps, core_ids=list(range(NCORES)))
    out = np.concatenate([res.results[k]["out"] for k in range(NCORES)], axis=0)
    return out.astype(np.float32)


if __name__ == "__main__":
    data = dict(np.load("/root/problem/inputs_cache.npz"))
    out = kernel(**data)
    exp = np.load("/root/problem/expected_np.npy")
    rel = np.linalg.norm(out - exp) / np.linalg.norm(exp)
    print("rel err:", rel)
